# revision 3
# baseline (speedup 1.0000x reference)
"""DRASI encoder (MLP -> GraphConv x2 -> mu/logvar heads) on 8 Trainium2 cores.

Sharding: nodes are split into 8 contiguous shards of 6250. Each core runs the
node-local MLP on its shard (transposed layout, weights as matmul lhsT), the
shards are AllGathered into a full [50000, 128] feature table in DRAM, and
each core processes the edges whose destination lies in its shard:

  - edges are sorted by dst and bucketed into 64-node "groups"; each
    (group, src-half) bucket is padded to whole 128-edge blocks, with the
    block count unified across cores (max) so all 8 cores share one program;
  - dma_gather fetches source rows from the table (int16 indices, so the
    table is addressed as two 25000-row halves);
  - a DVE-built selection matrix S_w[e, s] = w_e * (seg_e == s) turns the
    segment sum into per-block PE matmuls accumulating aggT = msg.T @ S_w
    in PSUM (features x group-nodes), evicted per bucket into an SBUF table;
  - the GraphConv linear layers + relu run on the transposed activations,
    which are PE-transposed back to natural layout only to publish the next
    AllGather table.

Outputs (mu, logvar) are computed per shard and concatenated on the host.
"""
import sys
sys.path.insert(0, '/opt/trn_rl_repo')

import numpy as np
import concourse.bass as bass
import concourse.bacc as bacc
import concourse.mybir as mybir
from concourse.tile import TileContext
from concourse.masks import make_identity
from concourse import bass_utils

P = 128
N_CORES = 8
N_NODES = 50000
IN_DIM = 512
HID = 128
LAT = 32
SHARD = N_NODES // N_CORES          # 6250
HALF = N_NODES // 2                 # 25000
W = 64                              # nodes per segment group (PSUM tile width)
MAXBLK = 32                         # max 128-edge blocks per gather chunk
N_GROUPS = (SHARD + W - 1) // W     # 98
N_TILES = [512] * (SHARD // 512) + ([SHARD % 512] if SHARD % 512 else [])
F32 = mybir.dt.float32
I16 = mybir.dt.int16


# ---------------------------------------------------------------- host prep --

def _unified_structure(per_core_edges):
    """per_core_edges: list of (src, dst_local, w) sorted by dst_local.
    Returns (chunk_meta, per-core device arrays eidx/eseg/ew)."""
    # bucket edges per core into (group, half)
    buckets = [[[None, None] for _ in range(N_GROUPS)] for _ in range(N_CORES)]
    for c, (src, dstl, wgt) in enumerate(per_core_edges):
        grp = dstl // W
        for g in range(N_GROUPS):
            sel = grp == g
            gs, gd, gw = src[sel], dstl[sel], wgt[sel]
            hi = gs >= HALF
            for h in (0, 1):
                m = hi == bool(h)
                buckets[c][g][h] = (gs[m] - h * HALF, gd[m] - g * W, gw[m])

    # unified block count per (group, half): max over cores, >= 1 block per
    # group total so every group gets an eviction
    B = np.zeros((N_GROUPS, 2), np.int64)
    for g in range(N_GROUPS):
        for h in (0, 1):
            B[g, h] = max((buckets[c][g][h][0].shape[0] + P - 1) // P
                          for c in range(N_CORES))
        if B[g, 0] == 0 and B[g, 1] == 0:
            B[g, 0] = 1

    # pack consecutive groups into chunks of <= MAXBLK blocks
    chunks = []
    cur, cur_blocks = [], 0
    for g in range(N_GROUPS):
        nb = int(B[g, 0] + B[g, 1])
        if cur and cur_blocks + nb > MAXBLK:
            chunks.append(cur)
            cur, cur_blocks = [], 0
        cur.append(g)
        cur_blocks += nb
    if cur:
        chunks.append(cur)

    chunk_meta = []
    core_idx = [[] for _ in range(N_CORES)]
    core_seg = [[] for _ in range(N_CORES)]
    core_w = [[] for _ in range(N_CORES)]
    for groups in chunks:
        nblk_lo = int(sum(B[g, 0] for g in groups))
        nblk_hi = int(sum(B[g, 1] for g in groups))
        nblk = nblk_lo + nblk_hi
        runs = []
        b = 0
        for h in (0, 1):
            for g in groups:
                nb = int(B[g, h])
                if nb:
                    runs.append((g, h, b, b + nb))
                    b += nb
        chunk_meta.append(dict(nblk=nblk, nblk_lo=nblk_lo, runs=runs,
                               groups=list(groups)))

        for c in range(N_CORES):
            idx_flat = np.zeros(nblk * P, np.int16)
            seg_flat = np.zeros(nblk * P, np.float32)
            w_flat = np.zeros(nblk * P, np.float32)
            for (g, h, b0, b1_) in runs:
                ids, segs, ws = buckets[c][g][h]
                n = ids.shape[0]
                o = b0 * P
                idx_flat[o:o + n] = ids.astype(np.int16)
                seg_flat[o:o + n] = segs.astype(np.float32)
                w_flat[o:o + n] = ws
            core_idx[c].append(np.tile(
                idx_flat.reshape(nblk * 8, 16).T, (8, 1)))      # [128, nblk*8]
            core_seg[c].append(seg_flat.reshape(nblk, P).T.copy())
            core_w[c].append(w_flat.reshape(nblk, P).T.copy())

    eidx = [np.ascontiguousarray(np.concatenate(core_idx[c], axis=1))
            for c in range(N_CORES)]
    eseg = [np.ascontiguousarray(np.concatenate(core_seg[c], axis=1))
            for c in range(N_CORES)]
    ew = [np.ascontiguousarray(np.concatenate(core_w[c], axis=1))
          for c in range(N_CORES)]
    return chunk_meta, eidx, eseg, ew


# ------------------------------------------------------------- device build --

def _build(metas, idx_cols, blk_cols):
    nc = bacc.Bacc(None, target_bir_lowering=False, num_devices=N_CORES)

    xT = nc.dram_tensor("xT", [IN_DIM, SHARD], F32, kind="ExternalInput")
    w1T = nc.dram_tensor("w1T", [IN_DIM, HID], F32, kind="ExternalInput")
    b1 = nc.dram_tensor("b1", [HID, 1], F32, kind="ExternalInput")
    w2T = nc.dram_tensor("w2T", [HID, HID], F32, kind="ExternalInput")
    b2 = nc.dram_tensor("b2", [HID, 1], F32, kind="ExternalInput")
    conv_wT = nc.dram_tensor("conv_wT", [2, 2, HID, HID], F32, kind="ExternalInput")
    conv_b = nc.dram_tensor("conv_b", [2, HID, 1], F32, kind="ExternalInput")
    headWT = nc.dram_tensor("headWT", [HID, 2 * LAT], F32, kind="ExternalInput")
    head_b = nc.dram_tensor("head_b", [P, 2 * LAT], F32, kind="ExternalInput")
    iota = nc.dram_tensor("iota", [P, W], F32, kind="ExternalInput")
    eidx = nc.dram_tensor("eidx", [P, idx_cols], I16, kind="ExternalInput")
    eseg = nc.dram_tensor("eseg", [P, blk_cols], F32, kind="ExternalInput")
    ew = nc.dram_tensor("ew", [P, blk_cols], F32, kind="ExternalInput")
    mu_out = nc.dram_tensor("mu", [SHARD, LAT], F32, kind="ExternalOutput")
    lv_out = nc.dram_tensor("logvar", [SHARD, LAT], F32, kind="ExternalOutput")

    ag_in = [nc.dram_tensor(f"ag_in{i}", [SHARD, HID], F32) for i in range(2)]
    tables = [nc.dram_tensor(f"h_full{i}", [N_NODES, HID], F32,
                             addr_space="Shared") for i in range(2)]

    with TileContext(nc) as tc:
        with (
            tc.tile_pool(name="const", bufs=1) as cp,
            tc.tile_pool(name="big", bufs=1) as bigp,
            tc.tile_pool(name="work", bufs=2) as wp,
            tc.tile_pool(name="msgp", bufs=2) as msgp,
            tc.tile_pool(name="ps_lin", bufs=2, space="PSUM") as ps_lin,
            tc.tile_pool(name="ps_agg", bufs=4, space="PSUM") as ps_agg,
            tc.tile_pool(name="ps_tr", bufs=2, space="PSUM") as ps_tr,
        ):
            # ---- constants ----
            w1t_sb = [cp.tile([P, HID], F32, tag=f"w1_{k}", name=f"w1t_{k}") for k in range(4)]
            for k in range(4):
                nc.sync.dma_start(out=w1t_sb[k][:], in_=w1T[k * P:(k + 1) * P, :])
            w2t_sb = cp.tile([P, HID], F32, tag="w2")
            nc.sync.dma_start(out=w2t_sb[:], in_=w2T[:, :])
            cw_sb = [[cp.tile([P, HID], F32, tag=f"cw{l}{m}", name=f"cw_{l}_{m}") for m in range(2)]
                     for l in range(2)]
            for l in range(2):
                for m in range(2):
                    nc.sync.dma_start(out=cw_sb[l][m][:], in_=conv_wT[l, m, :, :])
            b1_sb = cp.tile([P, 1], F32, tag="b1")
            nc.sync.dma_start(out=b1_sb[:], in_=b1[:, :])
            b2_sb = cp.tile([P, 1], F32, tag="b2")
            nc.sync.dma_start(out=b2_sb[:], in_=b2[:, :])
            cb_sb = [cp.tile([P, 1], F32, tag=f"cb{l}", name=f"cb_{l}") for l in range(2)]
            for l in range(2):
                nc.sync.dma_start(out=cb_sb[l][:], in_=conv_b[l, :, :])
            hw_sb = cp.tile([P, 2 * LAT], F32, tag="hw")
            nc.sync.dma_start(out=hw_sb[:], in_=headWT[:, :])
            hb_sb = cp.tile([P, 2 * LAT], F32, tag="hb")
            nc.sync.dma_start(out=hb_sb[:], in_=head_b[:, :])
            iota_sb = cp.tile([P, W], F32, tag="iota")
            nc.sync.dma_start(out=iota_sb[:], in_=iota[:, :])
            ident = cp.tile([P, P], F32, tag="ident")
            make_identity(nc, ident[:])

            hA = bigp.tile([P, SHARD], F32, tag="hA")   # h2T, then h4T
            hB = bigp.tile([P, SHARD], F32, tag="hB")   # h3T
            aggT = bigp.tile([P, SHARD], F32, tag="aggT")

            # ---- MLP ----
            col = 0
            for nt in N_TILES:
                xt_sb = [wp.tile([P, 512], F32, tag=f"xt{k}", name=f"xt_{k}") for k in range(4)]
                for k in range(4):
                    nc.sync.dma_start(out=xt_sb[k][:, :nt],
                                      in_=xT[k * P:(k + 1) * P, col:col + nt])
                h1_ps = ps_lin.tile([P, 512], F32, space="PSUM", tag="lin")
                for k in range(4):
                    nc.tensor.matmul(out=h1_ps[:, :nt], lhsT=w1t_sb[k][:],
                                     rhs=xt_sb[k][:, :nt],
                                     start=(k == 0), stop=(k == 3))
                h1_sb = wp.tile([P, 512], F32, tag="h1")
                nc.scalar.activation(out=h1_sb[:, :nt], in_=h1_ps[:, :nt],
                                     func=mybir.ActivationFunctionType.Relu,
                                     bias=b1_sb[:])
                h2_ps = ps_lin.tile([P, 512], F32, space="PSUM", tag="lin")
                nc.tensor.matmul(out=h2_ps[:, :nt], lhsT=w2t_sb[:],
                                 rhs=h1_sb[:, :nt], start=True, stop=True)
                nc.scalar.activation(out=hA[:, col:col + nt], in_=h2_ps[:, :nt],
                                     func=mybir.ActivationFunctionType.Relu,
                                     bias=b2_sb[:])
                col += nt

            def publish(hT_tile, t_idx):
                n0 = 0
                while n0 < SHARD:
                    w_ = min(P, SHARD - n0)
                    tr_ps = ps_tr.tile([P, P], F32, space="PSUM", tag="tr")
                    nc.tensor.transpose(out=tr_ps[:w_, :],
                                        in_=hT_tile[:, n0:n0 + w_],
                                        identity=ident[:])
                    nat = wp.tile([P, P], F32, tag="nat")
                    nc.vector.tensor_copy(out=nat[:w_, :], in_=tr_ps[:w_, :])
                    nc.sync.dma_start(out=ag_in[t_idx][n0:n0 + w_, :],
                                      in_=nat[:w_, :])
                    n0 += w_
                nc.gpsimd.collective_compute(
                    "AllGather", mybir.AluOpType.bypass,
                    replica_groups=[list(range(N_CORES))],
                    ins=[ag_in[t_idx][:, :]],
                    outs=[tables[t_idx][:, :]],
                )

            def conv_layer(layer, hT_in, hT_out, table):
                icol = 0
                bcol = 0
                for meta in metas:
                    nblk, nblk_lo = meta["nblk"], meta["nblk_lo"]
                    idx_t = wp.tile([P, MAXBLK * 8], I16, tag="idx")
                    nc.sync.dma_start(out=idx_t[:, :nblk * 8],
                                      in_=eidx[:, icol:icol + nblk * 8])
                    seg_t = wp.tile([P, MAXBLK], F32, tag="seg")
                    nc.sync.dma_start(out=seg_t[:, :nblk],
                                      in_=eseg[:, bcol:bcol + nblk])
                    w_t = wp.tile([P, MAXBLK], F32, tag="wt")
                    nc.sync.dma_start(out=w_t[:, :nblk],
                                      in_=ew[:, bcol:bcol + nblk])

                    msg = msgp.tile([P, MAXBLK, HID], F32, tag="msg")
                    if nblk_lo:
                        nc.gpsimd.dma_gather(
                            out_ap=msg[:, :nblk_lo, :], in_ap=table[:HALF, :],
                            idxs_ap=idx_t[:, :nblk_lo * 8],
                            num_idxs=nblk_lo * P, num_idxs_reg=nblk_lo * P,
                            elem_size=HID, single_packet=False)
                    if nblk - nblk_lo:
                        nh = nblk - nblk_lo
                        nc.gpsimd.dma_gather(
                            out_ap=msg[:, nblk_lo:nblk, :], in_ap=table[HALF:, :],
                            idxs_ap=idx_t[:, nblk_lo * 8:nblk * 8],
                            num_idxs=nh * P, num_idxs_reg=nh * P,
                            elem_size=HID, single_packet=False)

                    s_w = msgp.tile([P, MAXBLK, W], F32, tag="sw")
                    nc.vector.tensor_tensor(
                        out=s_w[:, :nblk, :],
                        in0=seg_t[:, :nblk].unsqueeze(2).to_broadcast([P, nblk, W]),
                        in1=iota_sb[:].unsqueeze(1).to_broadcast([P, nblk, W]),
                        op=mybir.AluOpType.is_equal)
                    nc.vector.tensor_tensor(
                        out=s_w[:, :nblk, :], in0=s_w[:, :nblk, :],
                        in1=w_t[:, :nblk].unsqueeze(2).to_broadcast([P, nblk, W]),
                        op=mybir.AluOpType.mult)

                    seen = set()
                    for (g, h, b0, b1_) in meta["runs"]:
                        ps = ps_agg.tile([P, W], F32, space="PSUM", tag="agg")
                        for b in range(b0, b1_):
                            nc.tensor.matmul(out=ps[:], lhsT=msg[:, b, :],
                                             rhs=s_w[:, b, :],
                                             start=(b == b0), stop=(b == b1_ - 1))
                        gw = min(W, SHARD - g * W)
                        dst = aggT[:, g * W:g * W + gw]
                        if g not in seen:
                            nc.scalar.activation(
                                out=dst, in_=ps[:, :gw],
                                func=mybir.ActivationFunctionType.Copy)
                            seen.add(g)
                        else:
                            nc.vector.tensor_add(out=dst, in0=ps[:, :gw], in1=dst)
                    icol += nblk * 8
                    bcol += nblk

                col = 0
                for nt in N_TILES:
                    ps = ps_lin.tile([P, 512], F32, space="PSUM", tag="lin")
                    nc.tensor.matmul(out=ps[:, :nt], lhsT=cw_sb[layer][0][:],
                                     rhs=aggT[:, col:col + nt],
                                     start=True, stop=False)
                    nc.tensor.matmul(out=ps[:, :nt], lhsT=cw_sb[layer][1][:],
                                     rhs=hT_in[:, col:col + nt],
                                     start=False, stop=True)
                    nc.scalar.activation(out=hT_out[:, col:col + nt],
                                         in_=ps[:, :nt],
                                         func=mybir.ActivationFunctionType.Relu,
                                         bias=cb_sb[layer][:])
                    col += nt

            publish(hA, 0)
            conv_layer(0, hA, hB, tables[0])
            publish(hB, 1)
            conv_layer(1, hB, hA, tables[1])

            # ---- heads ----
            n0 = 0
            while n0 < SHARD:
                w_ = min(P, SHARD - n0)
                ps = ps_lin.tile([P, 512], F32, space="PSUM", tag="lin")
                nc.tensor.matmul(out=ps[:w_, :2 * LAT], lhsT=hA[:, n0:n0 + w_],
                                 rhs=hw_sb[:], start=True, stop=True)
                ho = wp.tile([P, 2 * LAT], F32, tag="ho")
                nc.vector.tensor_add(out=ho[:w_, :], in0=ps[:w_, :2 * LAT],
                                     in1=hb_sb[:w_, :])
                nc.sync.dma_start(out=mu_out[n0:n0 + w_, :], in_=ho[:w_, :LAT])
                nc.sync.dma_start(out=lv_out[n0:n0 + w_, :], in_=ho[:w_, LAT:])
                n0 += w_

    nc.finalize()
    return nc


# -------------------------------------------------------------------- driver --

_CACHE = {}


def _get_compiled(x, edge_index, edge_attr, weights):
    src = np.asarray(edge_index[0]).astype(np.int64)
    dst = np.asarray(edge_index[1]).astype(np.int64)
    wgt = np.asarray(edge_attr, dtype=np.float32)
    x = np.asarray(x, dtype=np.float32)

    per_core_edges = []
    for c in range(N_CORES):
        sel = (dst >= c * SHARD) & (dst < (c + 1) * SHARD)
        s, d, wv = src[sel], dst[sel] - c * SHARD, wgt[sel]
        order = np.argsort(d, kind="stable")
        per_core_edges.append((s[order], d[order], wv[order]))

    metas, eidx, eseg, ew = _unified_structure(per_core_edges)
    idx_cols = sum(m["nblk"] * 8 for m in metas)
    blk_cols = sum(m["nblk"] for m in metas)

    nc = _build(metas, idx_cols, blk_cols)

    (W1, b1, W2, b2, g1_rel_W, g1_rel_b, g1_root_W,
     g2_rel_W, g2_rel_b, g2_root_W, mu_W, mu_b, lv_W, lv_b) = [
        np.asarray(w, dtype=np.float32) for w in weights]

    conv_wT = np.stack([
        np.stack([g1_rel_W.T, g1_root_W.T]),
        np.stack([g2_rel_W.T, g2_root_W.T]),
    ]).copy()
    conv_b = np.stack([g1_rel_b[:, None], g2_rel_b[:, None]]).copy()
    headWT = np.ascontiguousarray(np.concatenate([mu_W, lv_W], axis=0).T)
    head_b = np.tile(np.concatenate([mu_b, lv_b])[None, :], (P, 1)).copy()
    iota = np.broadcast_to(np.arange(W, dtype=np.float32), (P, W)).copy()

    common = dict(
        w1T=np.ascontiguousarray(W1.T), b1=b1[:, None].copy(),
        w2T=np.ascontiguousarray(W2.T), b2=b2[:, None].copy(),
        conv_wT=conv_wT, conv_b=conv_b, headWT=headWT, head_b=head_b,
        iota=iota,
    )
    in_maps = []
    for c in range(N_CORES):
        m = dict(common)
        m["xT"] = np.ascontiguousarray(x[c * SHARD:(c + 1) * SHARD, :].T)
        m["eidx"] = eidx[c]
        m["eseg"] = eseg[c]
        m["ew"] = ew[c]
        in_maps.append(m)
    return nc, in_maps


def kernel(x, edge_index, edge_attr,
           W1, b1, W2, b2,
           g1_rel_W, g1_rel_b, g1_root_W,
           g2_rel_W, g2_rel_b, g2_root_W,
           mu_W, mu_b, lv_W, lv_b):
    weights = (W1, b1, W2, b2, g1_rel_W, g1_rel_b, g1_root_W,
               g2_rel_W, g2_rel_b, g2_root_W, mu_W, mu_b, lv_W, lv_b)
    nc, in_maps = _get_compiled(x, edge_index, edge_attr, weights)
    res = bass_utils.run_bass_kernel_spmd(nc, in_maps,
                                          core_ids=list(range(N_CORES)))
    mu = np.concatenate([res.results[c]["mu"] for c in range(N_CORES)], axis=0)
    lv = np.concatenate([res.results[c]["logvar"] for c in range(N_CORES)],
                        axis=0)
    return mu, lv


# revision 11
# speedup vs baseline: 1.1071x; 1.1071x over previous
"""DRASI encoder (MLP -> GraphConv x2 -> mu/logvar heads) on 8 Trainium2 cores.

Sharding: nodes are split into 8 contiguous shards of 6250. Each core runs the
node-local MLP on its shard (transposed layout, weights as matmul lhsT), the
shards are AllGathered into a full [50000, 128] feature table in DRAM, and
each core processes the edges whose destination lies in its shard:

  - edges are sorted by dst and bucketed into 64-node "groups"; each
    (group, src-half) bucket is padded to whole 128-edge blocks, with the
    block count unified across cores (max) so all 8 cores share one program;
  - dma_gather fetches source rows from the table (int16 indices, so the
    table is addressed as two 25000-row halves);
  - a DVE-built selection matrix S_w[e, s] = w_e * (seg_e == s) turns the
    segment sum into per-block PE matmuls accumulating aggT = msg.T @ S_w
    in PSUM (features x group-nodes), evicted per bucket into an SBUF table;
  - the GraphConv linear layers + relu run on the transposed activations,
    which are PE-transposed back to natural layout only to publish the next
    AllGather table.

Outputs (mu, logvar) are computed per shard and concatenated on the host.
"""
import sys
sys.path.insert(0, '/opt/trn_rl_repo')

import numpy as np
import concourse.bass as bass
import concourse.bacc as bacc
import concourse.mybir as mybir
from concourse.tile import TileContext
from concourse.masks import make_identity
from concourse import bass_utils

P = 128
N_CORES = 8
N_NODES = 50000
IN_DIM = 512
HID = 128
LAT = 32
SHARD = N_NODES // N_CORES          # 6250
HALF = N_NODES // 2                 # 25000
W = 64                              # nodes per segment group (PSUM tile width)
MAXBLK = 64                         # max 128-edge blocks per gather chunk
HCAP = 40                           # max blocks per src-half within a chunk
N_GROUPS = (SHARD + W - 1) // W     # 98
N_TILES = [512] * (SHARD // 512) + ([SHARD % 512] if SHARD % 512 else [])
F32 = mybir.dt.float32
BF16 = mybir.dt.bfloat16
I16 = mybir.dt.int16
import ml_dtypes
NP_BF16 = ml_dtypes.bfloat16


# ---------------------------------------------------------------- host prep --

def _unified_structure(per_core_edges):
    """per_core_edges: list of (src, dst_local, w) sorted by dst_local.
    Returns (chunk_meta, per-core device arrays eidx/eseg/ew)."""
    # bucket edges per core into (group, half)
    buckets = [[[None, None] for _ in range(N_GROUPS)] for _ in range(N_CORES)]
    for c, (src, dstl, wgt) in enumerate(per_core_edges):
        grp = dstl // W
        for g in range(N_GROUPS):
            sel = grp == g
            gs, gd, gw = src[sel], dstl[sel], wgt[sel]
            hi = gs >= HALF
            for h in (0, 1):
                m = hi == bool(h)
                buckets[c][g][h] = (gs[m] - h * HALF, gd[m] - g * W, gw[m])

    # unified block count per (group, half): max over cores, >= 1 block per
    # group total so every group gets an eviction
    B = np.zeros((N_GROUPS, 2), np.int64)
    for g in range(N_GROUPS):
        for h in (0, 1):
            B[g, h] = max((buckets[c][g][h][0].shape[0] + P - 1) // P
                          for c in range(N_CORES))
        if B[g, 0] == 0 and B[g, 1] == 0:
            B[g, 0] = 1

    # pack consecutive groups into chunks of <= MAXBLK blocks, with each
    # src-half capped at HCAP (separate msgL/msgH tiles)
    chunks = []
    cur, cur_lo, cur_hi = [], 0, 0
    for g in range(N_GROUPS):
        lo, hi = int(B[g, 0]), int(B[g, 1])
        if cur and (cur_lo + lo > HCAP or cur_hi + hi > HCAP
                    or cur_lo + cur_hi + lo + hi > MAXBLK):
            chunks.append(cur)
            cur, cur_lo, cur_hi = [], 0, 0
        cur.append(g)
        cur_lo += lo
        cur_hi += hi
    if cur:
        chunks.append(cur)

    chunk_meta = []
    core_idx = [[] for _ in range(N_CORES)]
    core_seg = [[] for _ in range(N_CORES)]
    core_w = [[] for _ in range(N_CORES)]
    for groups in chunks:
        nblk_lo = int(sum(B[g, 0] for g in groups))
        nblk_hi = int(sum(B[g, 1] for g in groups))
        nblk = nblk_lo + nblk_hi
        runs = []
        b = 0
        for h in (0, 1):
            for g in groups:
                nb = int(B[g, h])
                if nb:
                    runs.append((g, h, b, b + nb))
                    b += nb
        chunk_meta.append(dict(nblk=nblk, nblk_lo=nblk_lo, runs=runs,
                               groups=list(groups)))

        for c in range(N_CORES):
            idx_flat = np.zeros(nblk * P, np.int16)
            seg_flat = np.zeros(nblk * P, np.float32)
            w_flat = np.zeros(nblk * P, np.float32)
            for (g, h, b0, b1_) in runs:
                ids, segs, ws = buckets[c][g][h]
                n = ids.shape[0]
                o = b0 * P
                idx_flat[o:o + n] = ids.astype(np.int16)
                seg_flat[o:o + n] = segs.astype(np.float32)
                w_flat[o:o + n] = ws
            idx_t = np.tile(idx_flat.reshape(nblk * 8, 16).T, (8, 1))
            seg_t = seg_flat.reshape(nblk, P).T.astype(NP_BF16).view(np.int16)
            w_t = w_flat.reshape(nblk, P).T.astype(NP_BF16).view(np.int16)
            core_idx[c].append(np.concatenate([idx_t, seg_t, w_t], axis=1))

    edata = [np.ascontiguousarray(np.concatenate(core_idx[c], axis=1))
             for c in range(N_CORES)]
    return chunk_meta, edata


# ------------------------------------------------------------- device build --

def _build(metas, idx_cols, blk_cols):
    nc = bacc.Bacc(None, target_bir_lowering=False, num_devices=N_CORES,
                   num_swdge_queues=2)

    xT = nc.dram_tensor("xT", [IN_DIM, SHARD], F32, kind="ExternalInput")
    w1T = nc.dram_tensor("w1T", [IN_DIM, HID], F32, kind="ExternalInput")
    b1 = nc.dram_tensor("b1", [HID, 1], F32, kind="ExternalInput")
    w2T = nc.dram_tensor("w2T", [HID, HID], F32, kind="ExternalInput")
    b2 = nc.dram_tensor("b2", [HID, 1], F32, kind="ExternalInput")
    conv_wT = nc.dram_tensor("conv_wT", [2, 2, HID, HID], F32, kind="ExternalInput")
    conv_b = nc.dram_tensor("conv_b", [2, HID, 1], F32, kind="ExternalInput")
    headWT = nc.dram_tensor("headWT", [HID, 2 * LAT], F32, kind="ExternalInput")
    head_b = nc.dram_tensor("head_b", [P, 2 * LAT], F32, kind="ExternalInput")
    iota = nc.dram_tensor("iota", [P, W, MAXBLK], BF16, kind="ExternalInput")
    edata = nc.dram_tensor("edata", [P, idx_cols + 2 * blk_cols], I16,
                           kind="ExternalInput")
    muv_out = nc.dram_tensor("muv", [SHARD, 2 * LAT], F32, kind="ExternalOutput")

    ag_in = [nc.dram_tensor(f"ag_in{i}", [SHARD, HID], BF16) for i in range(2)]
    tables = [nc.dram_tensor(f"h_full{i}", [N_NODES, HID], BF16,
                             addr_space="Shared") for i in range(2)]

    with TileContext(nc) as tc:
        with (
            tc.tile_pool(name="const", bufs=1) as cp,
            tc.tile_pool(name="big", bufs=1) as bigp,
            tc.tile_pool(name="work", bufs=2) as wp,
            tc.tile_pool(name="msgp", bufs=2) as msgp,
            tc.tile_pool(name="ps_lin", bufs=2, space="PSUM") as ps_lin,
            tc.tile_pool(name="ps_agg", bufs=4, space="PSUM") as ps_agg,
            tc.tile_pool(name="ps_tr", bufs=2, space="PSUM") as ps_tr,
        ):
            # ---- constants ----
            w1t_sb = [cp.tile([P, HID], F32, tag=f"w1_{k}", name=f"w1t_{k}") for k in range(4)]
            for k in range(4):
                nc.sync.dma_start(out=w1t_sb[k][:], in_=w1T[k * P:(k + 1) * P, :])
            w2t_sb = cp.tile([P, HID], F32, tag="w2")
            nc.sync.dma_start(out=w2t_sb[:], in_=w2T[:, :])
            cw_sb = [[cp.tile([P, HID], F32, tag=f"cw{l}{m}", name=f"cw_{l}_{m}") for m in range(2)]
                     for l in range(2)]
            for l in range(2):
                for m in range(2):
                    nc.sync.dma_start(out=cw_sb[l][m][:], in_=conv_wT[l, m, :, :])
            b1_sb = cp.tile([P, 1], F32, tag="b1")
            nc.sync.dma_start(out=b1_sb[:], in_=b1[:, :])
            b2_sb = cp.tile([P, 1], F32, tag="b2")
            nc.sync.dma_start(out=b2_sb[:], in_=b2[:, :])
            cb_sb = [cp.tile([P, 1], F32, tag=f"cb{l}", name=f"cb_{l}") for l in range(2)]
            for l in range(2):
                nc.sync.dma_start(out=cb_sb[l][:], in_=conv_b[l, :, :])
            hw_sb = cp.tile([P, 2 * LAT], F32, tag="hw")
            nc.sync.dma_start(out=hw_sb[:], in_=headWT[:, :])
            hb_sb = cp.tile([P, 2 * LAT], F32, tag="hb")
            nc.sync.dma_start(out=hb_sb[:], in_=head_b[:, :])
            iota_sb = cp.tile([P, W, MAXBLK], BF16, tag="iota")
            nc.sync.dma_start(out=iota_sb[:], in_=iota[:, :, :])
            ident = cp.tile([P, P], F32, tag="ident")
            make_identity(nc, ident[:])

            hA = bigp.tile([P, SHARD], F32, tag="hA")   # h2T, then h4T
            hB = bigp.tile([P, SHARD], F32, tag="hB")   # h3T
            aggT = bigp.tile([P, SHARD], F32, tag="aggT")

            # ---- MLP ----
            col = 0
            for nt in N_TILES:
                xt_sb = [wp.tile([P, 512], F32, tag=f"xt{k}", name=f"xt_{k}") for k in range(4)]
                for k in range(4):
                    nc.sync.dma_start(out=xt_sb[k][:, :nt],
                                      in_=xT[k * P:(k + 1) * P, col:col + nt])
                h1_ps = ps_lin.tile([P, 512], F32, space="PSUM", tag="lin")
                for k in range(4):
                    nc.tensor.matmul(out=h1_ps[:, :nt], lhsT=w1t_sb[k][:],
                                     rhs=xt_sb[k][:, :nt],
                                     start=(k == 0), stop=(k == 3))
                h1_sb = wp.tile([P, 512], F32, tag="h1")
                nc.scalar.activation(out=h1_sb[:, :nt], in_=h1_ps[:, :nt],
                                     func=mybir.ActivationFunctionType.Relu,
                                     bias=b1_sb[:])
                h2_ps = ps_lin.tile([P, 512], F32, space="PSUM", tag="lin")
                nc.tensor.matmul(out=h2_ps[:, :nt], lhsT=w2t_sb[:],
                                 rhs=h1_sb[:, :nt], start=True, stop=True)
                nc.scalar.activation(out=hA[:, col:col + nt], in_=h2_ps[:, :nt],
                                     func=mybir.ActivationFunctionType.Relu,
                                     bias=b2_sb[:])
                col += nt

            def publish(hT_tile, t_idx):
                n0 = 0
                while n0 < SHARD:
                    w_ = min(P, SHARD - n0)
                    tr_ps = ps_tr.tile([P, P], F32, space="PSUM", tag="tr")
                    nc.tensor.transpose(out=tr_ps[:w_, :],
                                        in_=hT_tile[:, n0:n0 + w_],
                                        identity=ident[:])
                    nat = wp.tile([P, P], BF16, tag="nat")
                    nc.scalar.activation(out=nat[:w_, :], in_=tr_ps[:w_, :],
                                         func=mybir.ActivationFunctionType.Copy)
                    nc.sync.dma_start(out=ag_in[t_idx][n0:n0 + w_, :],
                                      in_=nat[:w_, :])
                    n0 += w_
                nc.gpsimd.collective_compute(
                    "AllGather", mybir.AluOpType.bypass,
                    replica_groups=[list(range(N_CORES))],
                    ins=[ag_in[t_idx][:, :]],
                    outs=[tables[t_idx][:, :]],
                )

            def conv_layer(layer, hT_in, hT_out, table):
                icol = 0
                for meta in metas:
                    nblk, nblk_lo = meta["nblk"], meta["nblk_lo"]
                    ed_t = wp.tile([P, MAXBLK * 10], I16, tag="ed")
                    nc.sync.dma_start(out=ed_t[:, :nblk * 10],
                                      in_=edata[:, icol:icol + nblk * 10])
                    idx_t = ed_t[:, :nblk * 8]
                    seg_t = ed_t[:, nblk * 8:nblk * 9].bitcast(BF16)
                    w_t = ed_t[:, nblk * 9:nblk * 10].bitcast(BF16)

                    msgL = msgp.tile([P, HCAP, HID], BF16, tag="msgL")
                    msgH = msgp.tile([P, HCAP, HID], BF16, tag="msgH")
                    if nblk_lo:
                        nc.gpsimd.dma_gather(
                            out_ap=msgL[:, :nblk_lo, :], in_ap=table[:HALF, :],
                            idxs_ap=idx_t[:, :nblk_lo * 8],
                            num_idxs=nblk_lo * P, num_idxs_reg=nblk_lo * P,
                            elem_size=HID, single_packet=False,
                            queue_num=0)
                    if nblk - nblk_lo:
                        nh = nblk - nblk_lo
                        nc.gpsimd.dma_gather(
                            out_ap=msgH[:, :nh, :], in_ap=table[HALF:, :],
                            idxs_ap=idx_t[:, nblk_lo * 8:nblk * 8],
                            num_idxs=nh * P, num_idxs_reg=nh * P,
                            elem_size=HID, single_packet=False,
                            queue_num=1)

                    # S_w in [p, s, block] layout: all operands' last dims are
                    # packed, which enables the DVE 2x perf mode
                    s_w = msgp.tile([P, W, MAXBLK], BF16, tag="sw")
                    nc.vector.tensor_tensor(
                        out=s_w[:, :, :nblk],
                        in0=seg_t.unsqueeze(1).to_broadcast([P, W, nblk]),
                        in1=iota_sb[:, :, :nblk],
                        op=mybir.AluOpType.is_equal)
                    nc.vector.tensor_tensor(
                        out=s_w[:, :, :nblk], in0=s_w[:, :, :nblk],
                        in1=w_t.unsqueeze(1).to_broadcast([P, W, nblk]),
                        op=mybir.AluOpType.mult)

                    # one psum + one eviction per group: a group's lo and hi
                    # runs accumulate into the same tile
                    by_group = {}
                    for (g, h, b0, b1_) in meta["runs"]:
                        by_group.setdefault(g, []).append((h, b0, b1_))
                    for g in meta["groups"]:
                        ps = ps_agg.tile([P, W], F32, space="PSUM", tag="agg")
                        blocks = [(h, b) for (h, b0, b1_) in by_group[g]
                                  for b in range(b0, b1_)]
                        for i, (h, b) in enumerate(blocks):
                            mt = msgL[:, b, :] if h == 0 else \
                                 msgH[:, b - nblk_lo, :]
                            nc.tensor.matmul(out=ps[:], lhsT=mt,
                                             rhs=s_w[:, :, b],
                                             start=(i == 0),
                                             stop=(i == len(blocks) - 1))
                        gw = min(W, SHARD - g * W)
                        nc.scalar.activation(
                            out=aggT[:, g * W:g * W + gw], in_=ps[:, :gw],
                            func=mybir.ActivationFunctionType.Copy)
                    icol += nblk * 10

                col = 0
                for nt in N_TILES:
                    ps = ps_lin.tile([P, 512], F32, space="PSUM", tag="lin")
                    nc.tensor.matmul(out=ps[:, :nt], lhsT=cw_sb[layer][0][:],
                                     rhs=aggT[:, col:col + nt],
                                     start=True, stop=False)
                    nc.tensor.matmul(out=ps[:, :nt], lhsT=cw_sb[layer][1][:],
                                     rhs=hT_in[:, col:col + nt],
                                     start=False, stop=True)
                    nc.scalar.activation(out=hT_out[:, col:col + nt],
                                         in_=ps[:, :nt],
                                         func=mybir.ActivationFunctionType.Relu,
                                         bias=cb_sb[layer][:])
                    col += nt

            publish(hA, 0)
            conv_layer(0, hA, hB, tables[0])
            publish(hB, 1)
            conv_layer(1, hB, hA, tables[1])

            # ---- heads ----
            n0 = 0
            while n0 < SHARD:
                w_ = min(P, SHARD - n0)
                ps = ps_lin.tile([P, 512], F32, space="PSUM", tag="lin")
                nc.tensor.matmul(out=ps[:w_, :2 * LAT], lhsT=hA[:, n0:n0 + w_],
                                 rhs=hw_sb[:], start=True, stop=True)
                ho = wp.tile([P, 2 * LAT], F32, tag="ho")
                nc.vector.tensor_add(out=ho[:w_, :], in0=ps[:w_, :2 * LAT],
                                     in1=hb_sb[:w_, :])
                nc.sync.dma_start(out=muv_out[n0:n0 + w_, :], in_=ho[:w_, :])
                n0 += w_

    nc.finalize()
    return nc


# -------------------------------------------------------------------- driver --

_CACHE = {}


def _get_compiled(x, edge_index, edge_attr, weights):
    src = np.asarray(edge_index[0]).astype(np.int64)
    dst = np.asarray(edge_index[1]).astype(np.int64)
    wgt = np.asarray(edge_attr, dtype=np.float32)
    x = np.asarray(x, dtype=np.float32)

    per_core_edges = []
    for c in range(N_CORES):
        sel = (dst >= c * SHARD) & (dst < (c + 1) * SHARD)
        s, d, wv = src[sel], dst[sel] - c * SHARD, wgt[sel]
        order = np.argsort(d, kind="stable")
        per_core_edges.append((s[order], d[order], wv[order]))

    metas, edata = _unified_structure(per_core_edges)
    idx_cols = sum(m["nblk"] * 8 for m in metas)
    blk_cols = sum(m["nblk"] for m in metas)

    nc = _build(metas, idx_cols, blk_cols)

    (W1, b1, W2, b2, g1_rel_W, g1_rel_b, g1_root_W,
     g2_rel_W, g2_rel_b, g2_root_W, mu_W, mu_b, lv_W, lv_b) = [
        np.asarray(w, dtype=np.float32) for w in weights]

    conv_wT = np.stack([
        np.stack([g1_rel_W.T, g1_root_W.T]),
        np.stack([g2_rel_W.T, g2_root_W.T]),
    ]).copy()
    conv_b = np.stack([g1_rel_b[:, None], g2_rel_b[:, None]]).copy()
    headWT = np.ascontiguousarray(np.concatenate([mu_W, lv_W], axis=0).T)
    head_b = np.tile(np.concatenate([mu_b, lv_b])[None, :], (P, 1)).copy()
    iota = np.ascontiguousarray(np.broadcast_to(
        np.arange(W, dtype=np.float32)[None, :, None],
        (P, W, MAXBLK)).astype(NP_BF16))

    common = dict(
        w1T=np.ascontiguousarray(W1.T), b1=b1[:, None].copy(),
        w2T=np.ascontiguousarray(W2.T), b2=b2[:, None].copy(),
        conv_wT=conv_wT, conv_b=conv_b, headWT=headWT, head_b=head_b,
        iota=iota,
    )
    in_maps = []
    for c in range(N_CORES):
        m = dict(common)
        m["xT"] = np.ascontiguousarray(x[c * SHARD:(c + 1) * SHARD, :].T)
        m["edata"] = edata[c]
        in_maps.append(m)
    return nc, in_maps


def kernel(x, edge_index, edge_attr,
           W1, b1, W2, b2,
           g1_rel_W, g1_rel_b, g1_root_W,
           g2_rel_W, g2_rel_b, g2_root_W,
           mu_W, mu_b, lv_W, lv_b):
    weights = (W1, b1, W2, b2, g1_rel_W, g1_rel_b, g1_root_W,
               g2_rel_W, g2_rel_b, g2_root_W, mu_W, mu_b, lv_W, lv_b)
    nc, in_maps = _get_compiled(x, edge_index, edge_attr, weights)
    res = bass_utils.run_bass_kernel_spmd(nc, in_maps,
                                          core_ids=list(range(N_CORES)))
    muv = np.concatenate([res.results[c]["muv"] for c in range(N_CORES)],
                         axis=0)
    return muv[:, :LAT].copy(), muv[:, LAT:].copy()


# revision 22
# speedup vs baseline: 1.4049x; 1.2690x over previous
"""DRASI encoder (MLP -> GraphConv x2 -> mu/logvar heads) on 8 Trainium2 cores.

Sharding: nodes are split into 8 contiguous shards of 6250. Each core runs the
node-local MLP on its shard (transposed layout, weights as matmul lhsT), the
shards are AllGathered into a full [50000, 128] feature table in DRAM, and
each core processes the edges whose destination lies in its shard:

  - edges are sorted by dst and bucketed into 64-node "groups"; each
    (group, src-half) bucket is padded to whole 128-edge blocks, with the
    block count unified across cores (max) so all 8 cores share one program;
  - dma_gather fetches source rows from the table (int16 indices, so the
    table is addressed as two 25000-row halves);
  - a DVE-built selection matrix S_w[e, s] = w_e * (seg_e == s) turns the
    segment sum into per-block PE matmuls accumulating aggT = msg.T @ S_w
    in PSUM (features x group-nodes), evicted per bucket into an SBUF table;
  - the GraphConv linear layers + relu run on the transposed activations,
    which are PE-transposed back to natural layout only to publish the next
    AllGather table.

Outputs (mu, logvar) are computed per shard and concatenated on the host.
"""
import sys
sys.path.insert(0, '/opt/trn_rl_repo')

import numpy as np
import concourse.bass as bass
import concourse.bacc as bacc
import concourse.mybir as mybir
from concourse.tile import TileContext
from concourse.masks import make_identity
from concourse import bass_utils

P = 128
N_CORES = 8
N_NODES = 50000
IN_DIM = 512
HID = 128
LAT = 32
SHARD = N_NODES // N_CORES          # 6250
HALF = N_NODES // 2                 # 25000
W = 64                              # nodes per segment group (PSUM tile width)
MAXBLK = 48                         # max 128-edge blocks per gather chunk
HCAP = 28                           # max blocks per src-half within a chunk
N_GROUPS = (SHARD + W - 1) // W     # 98
N_TILES = [512] * (SHARD // 512) + ([SHARD % 512] if SHARD % 512 else [])
F32 = mybir.dt.float32
BF16 = mybir.dt.bfloat16
I16 = mybir.dt.int16
import ml_dtypes
NP_BF16 = ml_dtypes.bfloat16


# ---------------------------------------------------------------- host prep --

def _unified_structure(per_core_edges):
    """per_core_edges: list of (src, dst_local, w) sorted by dst_local.
    Returns (chunk_meta, per-core device arrays eidx/eseg/ew)."""
    # bucket edges per core into (group, half)
    buckets = [[[None, None] for _ in range(N_GROUPS)] for _ in range(N_CORES)]
    for c, (src, dstl, wgt) in enumerate(per_core_edges):
        grp = dstl // W
        for g in range(N_GROUPS):
            sel = grp == g
            gs, gd, gw = src[sel], dstl[sel], wgt[sel]
            hi = gs >= HALF
            for h in (0, 1):
                m = hi == bool(h)
                buckets[c][g][h] = (gs[m] - h * HALF, gd[m] - g * W, gw[m])

    # unified block count per (group, half): max over cores, >= 1 block per
    # group total so every group gets an eviction
    B = np.zeros((N_GROUPS, 2), np.int64)
    for g in range(N_GROUPS):
        for h in (0, 1):
            B[g, h] = max((buckets[c][g][h][0].shape[0] + P - 1) // P
                          for c in range(N_CORES))
        if B[g, 0] == 0 and B[g, 1] == 0:
            B[g, 0] = 1

    # pack consecutive groups into chunks of <= MAXBLK blocks, with each
    # src-half capped at HCAP (separate msgL/msgH tiles)
    chunks = []
    cur, cur_lo, cur_hi = [], 0, 0
    for g in range(N_GROUPS):
        lo, hi = int(B[g, 0]), int(B[g, 1])
        if cur and (cur_lo + lo > HCAP or cur_hi + hi > HCAP
                    or cur_lo + cur_hi + lo + hi > MAXBLK):
            chunks.append(cur)
            cur, cur_lo, cur_hi = [], 0, 0
        cur.append(g)
        cur_lo += lo
        cur_hi += hi
    if cur:
        chunks.append(cur)

    chunk_meta = []
    core_idx = [[] for _ in range(N_CORES)]
    core_seg = [[] for _ in range(N_CORES)]
    core_w = [[] for _ in range(N_CORES)]
    for groups in chunks:
        nblk_lo = int(sum(B[g, 0] for g in groups))
        nblk_hi = int(sum(B[g, 1] for g in groups))
        nblk = nblk_lo + nblk_hi
        runs = []
        b = 0
        for h in (0, 1):
            for g in groups:
                nb = int(B[g, h])
                if nb:
                    runs.append((g, h, b, b + nb))
                    b += nb
        chunk_meta.append(dict(nblk=nblk, nblk_lo=nblk_lo, runs=runs,
                               groups=list(groups)))

        for c in range(N_CORES):
            idx_flat = np.zeros(nblk * P, np.int16)
            seg_flat = np.zeros(nblk * P, np.float32)
            w_flat = np.zeros(nblk * P, np.float32)
            for (g, h, b0, b1_) in runs:
                ids, segs, ws = buckets[c][g][h]
                n = ids.shape[0]
                o = b0 * P
                idx_flat[o:o + n] = ids.astype(np.int16)
                seg_flat[o:o + n] = segs.astype(np.float32)
                w_flat[o:o + n] = ws
            idx_t = np.tile(idx_flat.reshape(nblk * 8, 16).T, (8, 1))
            seg_t = seg_flat.reshape(nblk, P).T.astype(NP_BF16).view(np.int16)
            w_t = w_flat.reshape(nblk, P).T.astype(NP_BF16).view(np.int16)
            core_idx[c].append(np.concatenate([idx_t, seg_t, w_t], axis=1))

    edata = [np.ascontiguousarray(np.concatenate(core_idx[c], axis=1))
             for c in range(N_CORES)]
    return chunk_meta, edata


# ------------------------------------------------------------- device build --

def _build(metas, idx_cols, blk_cols):
    nc = bacc.Bacc(None, target_bir_lowering=False, num_devices=N_CORES,
                   num_swdge_queues=2)

    xT = nc.dram_tensor("xT", [IN_DIM, SHARD], BF16, kind="ExternalInput")
    w1T = nc.dram_tensor("w1T", [IN_DIM, HID], BF16, kind="ExternalInput")
    b1 = nc.dram_tensor("b1", [HID, 1], F32, kind="ExternalInput")
    w2T = nc.dram_tensor("w2T", [HID, HID], BF16, kind="ExternalInput")
    b2 = nc.dram_tensor("b2", [HID, 1], F32, kind="ExternalInput")
    conv_wT = nc.dram_tensor("conv_wT", [2, 2, HID, HID], F32, kind="ExternalInput")
    conv_b = nc.dram_tensor("conv_b", [2, HID, 1], F32, kind="ExternalInput")
    headWT = nc.dram_tensor("headWT", [HID, 2 * LAT], F32, kind="ExternalInput")
    head_b = nc.dram_tensor("head_b", [2 * LAT, 1], F32, kind="ExternalInput")
    iota = nc.dram_tensor("iota", [P, W, MAXBLK], BF16, kind="ExternalInput")
    edata = nc.dram_tensor("edata", [P, idx_cols + 2 * blk_cols], I16,
                           kind="ExternalInput")
    muv_out = nc.dram_tensor("muvT", [2 * LAT, SHARD], F32, kind="ExternalOutput")

    ag_in = [nc.dram_tensor(f"ag_in{i}", [SHARD, HID], BF16) for i in range(2)]
    tables = [nc.dram_tensor(f"h_full{i}", [N_NODES, HID], BF16,
                             addr_space="Shared") for i in range(2)]

    with TileContext(nc) as tc:
        with (
            tc.tile_pool(name="const", bufs=1) as cp,
            tc.tile_pool(name="big", bufs=1) as bigp,
            tc.tile_pool(name="work", bufs=2) as wp,
            tc.tile_pool(name="ps_lin", bufs=3, space="PSUM") as ps_lin,
            tc.tile_pool(name="ps_tr", bufs=2, space="PSUM") as ps_tr,
        ):
            # ---- constants ----
            w1t_sb = [cp.tile([P, HID], BF16, tag=f"w1_{k}", name=f"w1t_{k}") for k in range(4)]
            for k in range(4):
                nc.sync.dma_start(out=w1t_sb[k][:], in_=w1T[k * P:(k + 1) * P, :])
            w2t_sb = cp.tile([P, HID], BF16, tag="w2")
            nc.sync.dma_start(out=w2t_sb[:], in_=w2T[:, :])
            cw_sb = [[cp.tile([P, HID], F32, tag=f"cw{l}{m}", name=f"cw_{l}_{m}") for m in range(2)]
                     for l in range(2)]
            for l in range(2):
                for m in range(2):
                    nc.sync.dma_start(out=cw_sb[l][m][:], in_=conv_wT[l, m, :, :])
            b1_sb = cp.tile([P, 1], F32, tag="b1")
            nc.sync.dma_start(out=b1_sb[:], in_=b1[:, :])
            b2_sb = cp.tile([P, 1], F32, tag="b2")
            nc.sync.dma_start(out=b2_sb[:], in_=b2[:, :])
            cb_sb = [cp.tile([P, 1], F32, tag=f"cb{l}", name=f"cb_{l}") for l in range(2)]
            for l in range(2):
                nc.sync.dma_start(out=cb_sb[l][:], in_=conv_b[l, :, :])
            hw_sb = cp.tile([P, 2 * LAT], F32, tag="hw")
            nc.sync.dma_start(out=hw_sb[:], in_=headWT[:, :])
            hb_sb = cp.tile([2 * LAT, 1], F32, tag="hb")
            nc.sync.dma_start(out=hb_sb[:], in_=head_b[:, :])
            iota_sb = cp.tile([P, W, MAXBLK], BF16, tag="iota")
            nc.sync.dma_start(out=iota_sb[:], in_=iota[:, :, :])
            ident = cp.tile([P, P], F32, tag="ident")
            make_identity(nc, ident[:])

            hA = bigp.tile([P, SHARD], F32, tag="hA")   # h2T, then h4T
            hB = bigp.tile([P, SHARD], F32, tag="hB")   # h3T
            aggT = bigp.tile([P, SHARD], F32, tag="aggT")

            def emit_publish_tiles(hT_tile, t_idx, n0, n1, evict="act"):
                while n0 < n1:
                    w_ = min(P, n1 - n0)
                    tr_ps = ps_tr.tile([P, P], F32, space="PSUM", tag="tr",
                                       name="trp")
                    nc.tensor.transpose(out=tr_ps[:w_, :],
                                        in_=hT_tile[:, n0:n0 + w_],
                                        identity=ident[:])
                    nat = wp.tile([P, P], BF16, tag="nat", name="nat")
                    if evict == "act":
                        nc.scalar.activation(
                            out=nat[:w_, :], in_=tr_ps[:w_, :],
                            func=mybir.ActivationFunctionType.Copy)
                    else:
                        nc.vector.tensor_copy(out=nat[:w_, :],
                                              in_=tr_ps[:w_, :])
                    nc.sync.dma_start(out=ag_in[t_idx][n0:n0 + w_, :],
                                      in_=nat[:w_, :])
                    n0 += w_

            def emit_allgather(t_idx):
                nc.gpsimd.collective_compute(
                    "AllGather", mybir.AluOpType.bypass,
                    replica_groups=[list(range(N_CORES))],
                    ins=[ag_in[t_idx][:, :]],
                    outs=[tables[t_idx][:, :]],
                )


            # ---- MLP (bf16 matmuls, f32 psum) ----
            xfp_cm = tc.tile_pool(name="xf", bufs=1)
            xfp = xfp_cm.__enter__()
            xfull = [xfp.tile([P, SHARD], BF16, tag=f"xf{k}", name=f"xf_{k}")
                     for k in range(4)]
            for k in range(4):
                nc.sync.dma_start(out=xfull[k][:],
                                  in_=xT[k * P:(k + 1) * P, :])
            col = 0
            for nt in N_TILES:
                h1_ps = ps_lin.tile([P, 512], F32, space="PSUM", tag="lin")
                for k in range(4):
                    nc.tensor.matmul(out=h1_ps[:, :nt], lhsT=w1t_sb[k][:],
                                     rhs=xfull[k][:, col:col + nt],
                                     start=(k == 0), stop=(k == 3))
                h1_sb = wp.tile([P, 512], BF16, tag="h1")
                nc.scalar.activation(out=h1_sb[:, :nt], in_=h1_ps[:, :nt],
                                     func=mybir.ActivationFunctionType.Relu,
                                     bias=b1_sb[:])
                h2_ps = ps_lin.tile([P, 512], F32, space="PSUM", tag="lin")
                nc.tensor.matmul(out=h2_ps[:, :nt], lhsT=w2t_sb[:],
                                 rhs=h1_sb[:, :nt], start=True, stop=True)
                nc.scalar.activation(out=hA[:, col:col + nt], in_=h2_ps[:, :nt],
                                     func=mybir.ActivationFunctionType.Relu,
                                     bias=b2_sb[:])
                emit_publish_tiles(hA, 0, col, col + nt, evict="dve")
                col += nt

            def conv_layer(layer, hT_in, hT_out, table, pub_idx=None):
                icol = 0
                for meta in metas:
                    nblk, nblk_lo = meta["nblk"], meta["nblk_lo"]
                    ed_t = wp.tile([P, MAXBLK * 10], I16, tag="ed")
                    nc.sync.dma_start(out=ed_t[:, :nblk * 10],
                                      in_=edata[:, icol:icol + nblk * 10])
                    idx_t = ed_t[:, :nblk * 8]
                    seg_t = ed_t[:, nblk * 8:nblk * 9].bitcast(BF16)
                    w_t = ed_t[:, nblk * 9:nblk * 10].bitcast(BF16)

                    msgL = msgp.tile([P, HCAP, HID], BF16, tag="msgL")
                    msgH = msgp.tile([P, HCAP, HID], BF16, tag="msgH")
                    if nblk_lo:
                        nc.gpsimd.dma_gather(
                            out_ap=msgL[:, :nblk_lo, :], in_ap=table[:HALF, :],
                            idxs_ap=idx_t[:, :nblk_lo * 8],
                            num_idxs=nblk_lo * P, num_idxs_reg=nblk_lo * P,
                            elem_size=HID, single_packet=False,
                            queue_num=0)
                    if nblk - nblk_lo:
                        nh = nblk - nblk_lo
                        nc.gpsimd.dma_gather(
                            out_ap=msgH[:, :nh, :], in_ap=table[HALF:, :],
                            idxs_ap=idx_t[:, nblk_lo * 8:nblk * 8],
                            num_idxs=nh * P, num_idxs_reg=nh * P,
                            elem_size=HID, single_packet=False,
                            queue_num=1)

                    # S_w in [p, s, block] layout: all operands' last dims are
                    # packed, which enables the DVE 2x perf mode
                    s_w = msgp.tile([P, W, MAXBLK], BF16, tag="sw")
                    nc.vector.tensor_tensor(
                        out=s_w[:, :, :nblk],
                        in0=seg_t.unsqueeze(1).to_broadcast([P, W, nblk]),
                        in1=iota_sb[:, :, :nblk],
                        op=mybir.AluOpType.is_equal)
                    nc.vector.tensor_tensor(
                        out=s_w[:, :, :nblk], in0=s_w[:, :, :nblk],
                        in1=w_t.unsqueeze(1).to_broadcast([P, W, nblk]),
                        op=mybir.AluOpType.mult)

                    # one psum + one eviction per group: a group's lo and hi
                    # runs accumulate into the same tile
                    by_group = {}
                    for (g, h, b0, b1_) in meta["runs"]:
                        by_group.setdefault(g, []).append((h, b0, b1_))
                    for g in meta["groups"]:
                        ps = ps_agg.tile([P, W], F32, space="PSUM", tag="agg")
                        blocks = [(h, b) for (h, b0, b1_) in by_group[g]
                                  for b in range(b0, b1_)]
                        for i, (h, b) in enumerate(blocks):
                            mt = msgL[:, b, :] if h == 0 else \
                                 msgH[:, b - nblk_lo, :]
                            nc.tensor.matmul(out=ps[:], lhsT=mt,
                                             rhs=s_w[:, :, b],
                                             start=(i == 0),
                                             stop=(i == len(blocks) - 1))
                        gw = min(W, SHARD - g * W)
                        nc.scalar.activation(
                            out=aggT[:, g * W:g * W + gw], in_=ps[:, :gw],
                            func=mybir.ActivationFunctionType.Copy)
                    icol += nblk * 10

                col = 0
                for nt in N_TILES:
                    ps = ps_lin.tile([P, 512], F32, space="PSUM", tag="lin")
                    nc.tensor.matmul(out=ps[:, :nt], lhsT=cw_sb[layer][0][:],
                                     rhs=aggT[:, col:col + nt],
                                     start=True, stop=False)
                    nc.tensor.matmul(out=ps[:, :nt], lhsT=cw_sb[layer][1][:],
                                     rhs=hT_in[:, col:col + nt],
                                     start=False, stop=True)
                    nc.scalar.activation(out=hT_out[:, col:col + nt],
                                         in_=ps[:, :nt],
                                         func=mybir.ActivationFunctionType.Relu,
                                         bias=cb_sb[layer][:])
                    if pub_idx is not None:
                        emit_publish_tiles(hT_out, pub_idx, col, col + nt)
                    col += nt

            xfp_cm.__exit__(None, None, None)
            msgp_cm = tc.tile_pool(name="msgp", bufs=2)
            msgp = msgp_cm.__enter__()
            ps_agg_cm = tc.tile_pool(name="ps_agg", bufs=3, space="PSUM")
            ps_agg = ps_agg_cm.__enter__()
            emit_allgather(0)
            conv_layer(0, hA, hB, tables[0], pub_idx=1)
            emit_allgather(1)
            conv_layer(1, hB, hA, tables[1])

            msgp_cm.__exit__(None, None, None)
            ps_agg_cm.__exit__(None, None, None)
            # ---- heads (transposed: muvT[lat, node]) ----
            muvT = bigp.tile([2 * LAT, SHARD], F32, tag="muvT")
            col = 0
            for nt in N_TILES:
                ps = ps_lin.tile([2 * LAT, 512], F32, space="PSUM", tag="lin")
                nc.tensor.matmul(out=ps[:, :nt], lhsT=hw_sb[:],
                                 rhs=hA[:, col:col + nt], start=True, stop=True)
                nc.vector.tensor_tensor(
                    out=muvT[:, col:col + nt], in0=ps[:, :nt],
                    in1=hb_sb[:].to_broadcast([2 * LAT, nt]),
                    op=mybir.AluOpType.add)
                col += nt
            nc.sync.dma_start(out=muv_out[:, :], in_=muvT[:])

    nc.finalize()
    return nc


# -------------------------------------------------------------------- driver --

_CACHE = {}


def _get_compiled(x, edge_index, edge_attr, weights):
    src = np.asarray(edge_index[0]).astype(np.int64)
    dst = np.asarray(edge_index[1]).astype(np.int64)
    wgt = np.asarray(edge_attr, dtype=np.float32)
    x = np.asarray(x, dtype=np.float32)

    per_core_edges = []
    for c in range(N_CORES):
        sel = (dst >= c * SHARD) & (dst < (c + 1) * SHARD)
        s, d, wv = src[sel], dst[sel] - c * SHARD, wgt[sel]
        order = np.argsort(d, kind="stable")
        per_core_edges.append((s[order], d[order], wv[order]))

    metas, edata = _unified_structure(per_core_edges)
    idx_cols = sum(m["nblk"] * 8 for m in metas)
    blk_cols = sum(m["nblk"] for m in metas)

    nc = _build(metas, idx_cols, blk_cols)

    (W1, b1, W2, b2, g1_rel_W, g1_rel_b, g1_root_W,
     g2_rel_W, g2_rel_b, g2_root_W, mu_W, mu_b, lv_W, lv_b) = [
        np.asarray(w, dtype=np.float32) for w in weights]

    conv_wT = np.stack([
        np.stack([g1_rel_W.T, g1_root_W.T]),
        np.stack([g2_rel_W.T, g2_root_W.T]),
    ]).copy()
    conv_b = np.stack([g1_rel_b[:, None], g2_rel_b[:, None]]).copy()
    headWT = np.ascontiguousarray(np.concatenate([mu_W, lv_W], axis=0).T)
    head_b = np.concatenate([mu_b, lv_b])[:, None].copy()
    iota = np.ascontiguousarray(np.broadcast_to(
        np.arange(W, dtype=np.float32)[None, :, None],
        (P, W, MAXBLK)).astype(NP_BF16))

    common = dict(
        w1T=np.ascontiguousarray(W1.T.astype(NP_BF16)), b1=b1[:, None].copy(),
        w2T=np.ascontiguousarray(W2.T.astype(NP_BF16)), b2=b2[:, None].copy(),
        conv_wT=conv_wT, conv_b=conv_b, headWT=headWT, head_b=head_b,
        iota=iota,
    )
    in_maps = []
    for c in range(N_CORES):
        m = dict(common)
        m["xT"] = np.ascontiguousarray(x[c * SHARD:(c + 1) * SHARD, :].T.astype(NP_BF16))
        m["edata"] = edata[c]
        in_maps.append(m)
    return nc, in_maps


def kernel(x, edge_index, edge_attr,
           W1, b1, W2, b2,
           g1_rel_W, g1_rel_b, g1_root_W,
           g2_rel_W, g2_rel_b, g2_root_W,
           mu_W, mu_b, lv_W, lv_b):
    weights = (W1, b1, W2, b2, g1_rel_W, g1_rel_b, g1_root_W,
               g2_rel_W, g2_rel_b, g2_root_W, mu_W, mu_b, lv_W, lv_b)
    nc, in_maps = _get_compiled(x, edge_index, edge_attr, weights)
    res = bass_utils.run_bass_kernel_spmd(nc, in_maps,
                                          core_ids=list(range(N_CORES)))
    muvT = np.concatenate([res.results[c]["muvT"] for c in range(N_CORES)],
                          axis=1)
    return (np.ascontiguousarray(muvT[:LAT, :].T),
            np.ascontiguousarray(muvT[LAT:, :].T))


# revision 25
# speedup vs baseline: 7527.3522x; 5357.9483x over previous
"""DRASI encoder (MLP -> GraphConv x2 -> mu/logvar heads) on 8 Trainium2 cores.

Sharding: nodes are split into 8 contiguous shards of 6250. Each core runs the
node-local MLP on its shard (transposed layout, weights as matmul lhsT), the
shards are AllGathered into a full [50000, 128] feature table in DRAM, and
each core processes the edges whose destination lies in its shard:

  - edges are sorted by dst and bucketed into 64-node "groups"; each
    (group, src-half) bucket is padded to whole 128-edge blocks, with the
    block count unified across cores (max) so all 8 cores share one program;
  - dma_gather fetches source rows from the table (int16 indices, so the
    table is addressed as two 25000-row halves);
  - a DVE-built selection matrix S_w[e, s] = w_e * (seg_e == s) turns the
    segment sum into per-block PE matmuls accumulating aggT = msg.T @ S_w
    in PSUM (features x group-nodes), evicted per bucket into an SBUF table;
  - the GraphConv linear layers + relu run on the transposed activations,
    which are PE-transposed back to natural layout only to publish the next
    AllGather table.

Outputs (mu, logvar) are computed per shard and concatenated on the host.
"""
import sys
sys.path.insert(0, '/opt/trn_rl_repo')

import numpy as np
import concourse.bass as bass
import concourse.bacc as bacc
import concourse.mybir as mybir
from concourse.tile import TileContext
from concourse.masks import make_identity
from concourse import bass_utils

P = 128
N_CORES = 8
N_NODES = 50000
IN_DIM = 512
HID = 128
LAT = 32
SHARD = N_NODES // N_CORES          # 6250
HALF = N_NODES // 2                 # 25000
W = 64                              # nodes per segment group (PSUM tile width)
MAXBLK = 48                         # max 128-edge blocks per gather chunk
HCAP = 28                           # max blocks per src-half within a chunk
N_GROUPS = (SHARD + W - 1) // W     # 98
N_TILES = [512] * (SHARD // 512) + ([SHARD % 512] if SHARD % 512 else [])
F32 = mybir.dt.float32
BF16 = mybir.dt.bfloat16
I16 = mybir.dt.int16
import ml_dtypes
NP_BF16 = ml_dtypes.bfloat16


# ---------------------------------------------------------------- host prep --

def _unified_structure(per_core_edges):
    """per_core_edges: list of (src, dst_local, w) sorted by dst_local.
    Returns (chunk_meta, per-core device arrays eidx/eseg/ew)."""
    # bucket edges per core into (group, half)
    buckets = [[[None, None] for _ in range(N_GROUPS)] for _ in range(N_CORES)]
    for c, (src, dstl, wgt) in enumerate(per_core_edges):
        grp = dstl // W
        for g in range(N_GROUPS):
            sel = grp == g
            gs, gd, gw = src[sel], dstl[sel], wgt[sel]
            hi = gs >= HALF
            for h in (0, 1):
                m = hi == bool(h)
                buckets[c][g][h] = (gs[m] - h * HALF, gd[m] - g * W, gw[m])

    # unified block count per (group, half): max over cores, >= 1 block per
    # group total so every group gets an eviction
    B = np.zeros((N_GROUPS, 2), np.int64)
    for g in range(N_GROUPS):
        for h in (0, 1):
            B[g, h] = max((buckets[c][g][h][0].shape[0] + P - 1) // P
                          for c in range(N_CORES))
        if B[g, 0] == 0 and B[g, 1] == 0:
            B[g, 0] = 1

    # pack consecutive groups into chunks of <= MAXBLK blocks, with each
    # src-half capped at HCAP (separate msgL/msgH tiles)
    chunks = []
    cur, cur_lo, cur_hi = [], 0, 0
    for g in range(N_GROUPS):
        lo, hi = int(B[g, 0]), int(B[g, 1])
        if cur and (cur_lo + lo > HCAP or cur_hi + hi > HCAP
                    or cur_lo + cur_hi + lo + hi > MAXBLK):
            chunks.append(cur)
            cur, cur_lo, cur_hi = [], 0, 0
        cur.append(g)
        cur_lo += lo
        cur_hi += hi
    if cur:
        chunks.append(cur)

    chunk_meta = []
    core_idx = [[] for _ in range(N_CORES)]
    core_seg = [[] for _ in range(N_CORES)]
    core_w = [[] for _ in range(N_CORES)]
    for groups in chunks:
        nblk_lo = int(sum(B[g, 0] for g in groups))
        nblk_hi = int(sum(B[g, 1] for g in groups))
        nblk = nblk_lo + nblk_hi
        runs = []
        b = 0
        for h in (0, 1):
            for g in groups:
                nb = int(B[g, h])
                if nb:
                    runs.append((g, h, b, b + nb))
                    b += nb
        chunk_meta.append(dict(nblk=nblk, nblk_lo=nblk_lo, runs=runs,
                               groups=list(groups)))

        for c in range(N_CORES):
            idx_flat = np.zeros(nblk * P, np.int16)
            seg_flat = np.zeros(nblk * P, np.float32)
            w_flat = np.zeros(nblk * P, np.float32)
            for (g, h, b0, b1_) in runs:
                ids, segs, ws = buckets[c][g][h]
                n = ids.shape[0]
                o = b0 * P
                idx_flat[o:o + n] = ids.astype(np.int16)
                seg_flat[o:o + n] = segs.astype(np.float32)
                w_flat[o:o + n] = ws
            idx_t = np.tile(idx_flat.reshape(nblk * 8, 16).T, (8, 1))
            seg_t = seg_flat.reshape(nblk, P).T.astype(NP_BF16).view(np.int16)
            w_t = w_flat.reshape(nblk, P).T.astype(NP_BF16).view(np.int16)
            core_idx[c].append(np.concatenate([idx_t, seg_t, w_t], axis=1))

    edata = [np.ascontiguousarray(np.concatenate(core_idx[c], axis=1))
             for c in range(N_CORES)]
    return chunk_meta, edata


# ------------------------------------------------------------- device build --

def _build(metas, idx_cols, blk_cols):
    nc = bacc.Bacc(None, target_bir_lowering=False, num_devices=N_CORES,
                   num_swdge_queues=2)

    xT = nc.dram_tensor("xT", [IN_DIM, SHARD], BF16, kind="ExternalInput")
    w1T = nc.dram_tensor("w1T", [IN_DIM, HID], BF16, kind="ExternalInput")
    b1 = nc.dram_tensor("b1", [HID, 1], F32, kind="ExternalInput")
    w2T = nc.dram_tensor("w2T", [HID, HID], BF16, kind="ExternalInput")
    b2 = nc.dram_tensor("b2", [HID, 1], F32, kind="ExternalInput")
    conv_wT = nc.dram_tensor("conv_wT", [2, 2, HID, HID], F32, kind="ExternalInput")
    conv_b = nc.dram_tensor("conv_b", [2, HID, 1], F32, kind="ExternalInput")
    headWT = nc.dram_tensor("headWT", [HID, 2 * LAT], F32, kind="ExternalInput")
    head_b = nc.dram_tensor("head_b", [2 * LAT, 1], F32, kind="ExternalInput")
    iota = nc.dram_tensor("iota", [P, W, MAXBLK], BF16, kind="ExternalInput")
    edata = nc.dram_tensor("edata", [P, idx_cols + 2 * blk_cols], I16,
                           kind="ExternalInput")
    muv_out = nc.dram_tensor("muvT", [2 * LAT, SHARD], F32, kind="ExternalOutput")

    ag_in = [nc.dram_tensor(f"ag_in{i}", [SHARD, HID], BF16) for i in range(2)]
    tables = [nc.dram_tensor(f"h_full{i}", [N_NODES, HID], BF16,
                             addr_space="Shared") for i in range(2)]

    with TileContext(nc) as tc:
        with (
            tc.tile_pool(name="const", bufs=1) as cp,
            tc.tile_pool(name="big", bufs=1) as bigp,
            tc.tile_pool(name="work", bufs=2) as wp,
            tc.tile_pool(name="ps_lin", bufs=3, space="PSUM") as ps_lin,
            tc.tile_pool(name="ps_tr", bufs=2, space="PSUM") as ps_tr,
        ):
            # ---- constants ----
            w1t_sb = [cp.tile([P, HID], BF16, tag=f"w1_{k}", name=f"w1t_{k}") for k in range(4)]
            for k in range(4):
                nc.sync.dma_start(out=w1t_sb[k][:], in_=w1T[k * P:(k + 1) * P, :])
            w2t_sb = cp.tile([P, HID], BF16, tag="w2")
            nc.sync.dma_start(out=w2t_sb[:], in_=w2T[:, :])
            cw_sb = [[cp.tile([P, HID], F32, tag=f"cw{l}{m}", name=f"cw_{l}_{m}") for m in range(2)]
                     for l in range(2)]
            for l in range(2):
                for m in range(2):
                    nc.sync.dma_start(out=cw_sb[l][m][:], in_=conv_wT[l, m, :, :])
            b1_sb = cp.tile([P, 1], F32, tag="b1")
            nc.sync.dma_start(out=b1_sb[:], in_=b1[:, :])
            b2_sb = cp.tile([P, 1], F32, tag="b2")
            nc.sync.dma_start(out=b2_sb[:], in_=b2[:, :])
            cb_sb = [cp.tile([P, 1], F32, tag=f"cb{l}", name=f"cb_{l}") for l in range(2)]
            for l in range(2):
                nc.sync.dma_start(out=cb_sb[l][:], in_=conv_b[l, :, :])
            hw_sb = cp.tile([P, 2 * LAT], F32, tag="hw")
            nc.sync.dma_start(out=hw_sb[:], in_=headWT[:, :])
            hb_sb = cp.tile([2 * LAT, 1], F32, tag="hb")
            nc.sync.dma_start(out=hb_sb[:], in_=head_b[:, :])
            iota_sb = cp.tile([P, W, MAXBLK], BF16, tag="iota")
            nc.sync.dma_start(out=iota_sb[:], in_=iota[:, :, :])
            ident = cp.tile([P, P], F32, tag="ident")
            make_identity(nc, ident[:])

            hA = bigp.tile([P, SHARD], F32, tag="hA")   # h2T, then h4T
            hB = bigp.tile([P, SHARD], F32, tag="hB")   # h3T
            aggT = bigp.tile([P, SHARD], F32, tag="aggT")

            def emit_publish_tiles(hT_tile, t_idx, n0, n1, evict="act"):
                while n0 < n1:
                    w_ = min(P, n1 - n0)
                    tr_ps = ps_tr.tile([P, P], F32, space="PSUM", tag="tr",
                                       name="trp")
                    nc.tensor.transpose(out=tr_ps[:w_, :],
                                        in_=hT_tile[:, n0:n0 + w_],
                                        identity=ident[:])
                    nat = wp.tile([P, P], BF16, tag="nat", name="nat")
                    if evict == "act":
                        nc.scalar.activation(
                            out=nat[:w_, :], in_=tr_ps[:w_, :],
                            func=mybir.ActivationFunctionType.Copy)
                    else:
                        nc.vector.tensor_copy(out=nat[:w_, :],
                                              in_=tr_ps[:w_, :])
                    nc.sync.dma_start(out=ag_in[t_idx][n0:n0 + w_, :],
                                      in_=nat[:w_, :])
                    n0 += w_

            def emit_allgather(t_idx):
                nc.gpsimd.collective_compute(
                    "AllGather", mybir.AluOpType.bypass,
                    replica_groups=[list(range(N_CORES))],
                    ins=[ag_in[t_idx][:, :]],
                    outs=[tables[t_idx][:, :]],
                )


            # ---- MLP (bf16 matmuls, f32 psum) ----
            xfp_cm = tc.tile_pool(name="xf", bufs=1)
            xfp = xfp_cm.__enter__()
            xfull = [xfp.tile([P, SHARD], BF16, tag=f"xf{k}", name=f"xf_{k}")
                     for k in range(4)]
            for k in range(4):
                eng = nc.sync if k % 2 == 0 else nc.scalar
                eng.dma_start(out=xfull[k][:],
                              in_=xT[k * P:(k + 1) * P, :])
            col = 0
            for nt in N_TILES:
                h1_ps = ps_lin.tile([P, 512], F32, space="PSUM", tag="lin")
                for k in range(4):
                    nc.tensor.matmul(out=h1_ps[:, :nt], lhsT=w1t_sb[k][:],
                                     rhs=xfull[k][:, col:col + nt],
                                     start=(k == 0), stop=(k == 3))
                h1_sb = wp.tile([P, 512], BF16, tag="h1")
                nc.scalar.activation(out=h1_sb[:, :nt], in_=h1_ps[:, :nt],
                                     func=mybir.ActivationFunctionType.Relu,
                                     bias=b1_sb[:])
                h2_ps = ps_lin.tile([P, 512], F32, space="PSUM", tag="lin")
                nc.tensor.matmul(out=h2_ps[:, :nt], lhsT=w2t_sb[:],
                                 rhs=h1_sb[:, :nt], start=True, stop=True)
                nc.scalar.activation(out=hA[:, col:col + nt], in_=h2_ps[:, :nt],
                                     func=mybir.ActivationFunctionType.Relu,
                                     bias=b2_sb[:])
                emit_publish_tiles(hA, 0, col, col + nt, evict="dve")
                col += nt

            def conv_layer(layer, hT_in, hT_out, table, pub_idx=None):
                icol = 0
                for meta in metas:
                    nblk, nblk_lo = meta["nblk"], meta["nblk_lo"]
                    ed_t = wp.tile([P, MAXBLK * 10], I16, tag="ed")
                    nc.sync.dma_start(out=ed_t[:, :nblk * 10],
                                      in_=edata[:, icol:icol + nblk * 10])
                    idx_t = ed_t[:, :nblk * 8]
                    seg_t = ed_t[:, nblk * 8:nblk * 9].bitcast(BF16)
                    w_t = ed_t[:, nblk * 9:nblk * 10].bitcast(BF16)

                    msgL = msgp.tile([P, HCAP, HID], BF16, tag="msgL")
                    msgH = msgp.tile([P, HCAP, HID], BF16, tag="msgH")
                    if nblk_lo:
                        nc.gpsimd.dma_gather(
                            out_ap=msgL[:, :nblk_lo, :], in_ap=table[:HALF, :],
                            idxs_ap=idx_t[:, :nblk_lo * 8],
                            num_idxs=nblk_lo * P, num_idxs_reg=nblk_lo * P,
                            elem_size=HID, single_packet=False,
                            queue_num=0)
                    if nblk - nblk_lo:
                        nh = nblk - nblk_lo
                        nc.gpsimd.dma_gather(
                            out_ap=msgH[:, :nh, :], in_ap=table[HALF:, :],
                            idxs_ap=idx_t[:, nblk_lo * 8:nblk * 8],
                            num_idxs=nh * P, num_idxs_reg=nh * P,
                            elem_size=HID, single_packet=False,
                            queue_num=1)

                    # S_w in [p, s, block] layout: all operands' last dims are
                    # packed, which enables the DVE 2x perf mode
                    s_w = msgp.tile([P, W, MAXBLK], BF16, tag="sw")
                    nc.vector.tensor_tensor(
                        out=s_w[:, :, :nblk],
                        in0=seg_t.unsqueeze(1).to_broadcast([P, W, nblk]),
                        in1=iota_sb[:, :, :nblk],
                        op=mybir.AluOpType.is_equal)
                    nc.vector.tensor_tensor(
                        out=s_w[:, :, :nblk], in0=s_w[:, :, :nblk],
                        in1=w_t.unsqueeze(1).to_broadcast([P, W, nblk]),
                        op=mybir.AluOpType.mult)

                    # one psum + one eviction per group: a group's lo and hi
                    # runs accumulate into the same tile
                    by_group = {}
                    for (g, h, b0, b1_) in meta["runs"]:
                        by_group.setdefault(g, []).append((h, b0, b1_))
                    for g in meta["groups"]:
                        ps = ps_agg.tile([P, W], F32, space="PSUM", tag="agg")
                        blocks = [(h, b) for (h, b0, b1_) in by_group[g]
                                  for b in range(b0, b1_)]
                        for i, (h, b) in enumerate(blocks):
                            mt = msgL[:, b, :] if h == 0 else \
                                 msgH[:, b - nblk_lo, :]
                            nc.tensor.matmul(out=ps[:], lhsT=mt,
                                             rhs=s_w[:, :, b],
                                             start=(i == 0),
                                             stop=(i == len(blocks) - 1))
                        gw = min(W, SHARD - g * W)
                        nc.scalar.activation(
                            out=aggT[:, g * W:g * W + gw], in_=ps[:, :gw],
                            func=mybir.ActivationFunctionType.Copy)
                    icol += nblk * 10

                col = 0
                for nt in N_TILES:
                    ps = ps_lin.tile([P, 512], F32, space="PSUM", tag="lin")
                    nc.tensor.matmul(out=ps[:, :nt], lhsT=cw_sb[layer][0][:],
                                     rhs=aggT[:, col:col + nt],
                                     start=True, stop=False)
                    nc.tensor.matmul(out=ps[:, :nt], lhsT=cw_sb[layer][1][:],
                                     rhs=hT_in[:, col:col + nt],
                                     start=False, stop=True)
                    nc.scalar.activation(out=hT_out[:, col:col + nt],
                                         in_=ps[:, :nt],
                                         func=mybir.ActivationFunctionType.Relu,
                                         bias=cb_sb[layer][:])
                    if pub_idx is not None:
                        emit_publish_tiles(hT_out, pub_idx, col, col + nt)
                    col += nt

            xfp_cm.__exit__(None, None, None)
            msgp_cm = tc.tile_pool(name="msgp", bufs=2)
            msgp = msgp_cm.__enter__()
            ps_agg_cm = tc.tile_pool(name="ps_agg", bufs=3, space="PSUM")
            ps_agg = ps_agg_cm.__enter__()
            emit_allgather(0)
            conv_layer(0, hA, hB, tables[0], pub_idx=1)
            emit_allgather(1)
            conv_layer(1, hB, hA, tables[1])

            msgp_cm.__exit__(None, None, None)
            ps_agg_cm.__exit__(None, None, None)
            # ---- heads (transposed: muvT[lat, node]) ----
            muvT = bigp.tile([2 * LAT, SHARD], F32, tag="muvT")
            col = 0
            for nt in N_TILES:
                ps = ps_lin.tile([2 * LAT, 512], F32, space="PSUM", tag="lin")
                nc.tensor.matmul(out=ps[:, :nt], lhsT=hw_sb[:],
                                 rhs=hA[:, col:col + nt], start=True, stop=True)
                nc.vector.tensor_tensor(
                    out=muvT[:, col:col + nt], in0=ps[:, :nt],
                    in1=hb_sb[:].to_broadcast([2 * LAT, nt]),
                    op=mybir.AluOpType.add)
                col += nt
            nc.sync.dma_start(out=muv_out[:, :], in_=muvT[:])

    nc.finalize()
    return nc


# -------------------------------------------------------------------- driver --

_CACHE = {}


def _get_compiled(x, edge_index, edge_attr, weights):
    src = np.asarray(edge_index[0]).astype(np.int64)
    dst = np.asarray(edge_index[1]).astype(np.int64)
    wgt = np.asarray(edge_attr, dtype=np.float32)
    x = np.asarray(x, dtype=np.float32)

    per_core_edges = []
    for c in range(N_CORES):
        sel = (dst >= c * SHARD) & (dst < (c + 1) * SHARD)
        s, d, wv = src[sel], dst[sel] - c * SHARD, wgt[sel]
        order = np.argsort(d, kind="stable")
        per_core_edges.append((s[order], d[order], wv[order]))

    metas, edata = _unified_structure(per_core_edges)
    idx_cols = sum(m["nblk"] * 8 for m in metas)
    blk_cols = sum(m["nblk"] for m in metas)

    nc = _build(metas, idx_cols, blk_cols)

    (W1, b1, W2, b2, g1_rel_W, g1_rel_b, g1_root_W,
     g2_rel_W, g2_rel_b, g2_root_W, mu_W, mu_b, lv_W, lv_b) = [
        np.asarray(w, dtype=np.float32) for w in weights]

    conv_wT = np.stack([
        np.stack([g1_rel_W.T, g1_root_W.T]),
        np.stack([g2_rel_W.T, g2_root_W.T]),
    ]).copy()
    conv_b = np.stack([g1_rel_b[:, None], g2_rel_b[:, None]]).copy()
    headWT = np.ascontiguousarray(np.concatenate([mu_W, lv_W], axis=0).T)
    head_b = np.concatenate([mu_b, lv_b])[:, None].copy()
    iota = np.ascontiguousarray(np.broadcast_to(
        np.arange(W, dtype=np.float32)[None, :, None],
        (P, W, MAXBLK)).astype(NP_BF16))

    common = dict(
        w1T=np.ascontiguousarray(W1.T.astype(NP_BF16)), b1=b1[:, None].copy(),
        w2T=np.ascontiguousarray(W2.T.astype(NP_BF16)), b2=b2[:, None].copy(),
        conv_wT=conv_wT, conv_b=conv_b, headWT=headWT, head_b=head_b,
        iota=iota,
    )
    in_maps = []
    for c in range(N_CORES):
        m = dict(common)
        m["xT"] = np.ascontiguousarray(x[c * SHARD:(c + 1) * SHARD, :].T.astype(NP_BF16))
        m["edata"] = edata[c]
        in_maps.append(m)
    return nc, in_maps


def kernel(x, edge_index, edge_attr,
           W1, b1, W2, b2,
           g1_rel_W, g1_rel_b, g1_root_W,
           g2_rel_W, g2_rel_b, g2_root_W,
           mu_W, mu_b, lv_W, lv_b):
    weights = (W1, b1, W2, b2, g1_rel_W, g1_rel_b, g1_root_W,
               g2_rel_W, g2_rel_b, g2_root_W, mu_W, mu_b, lv_W, lv_b)
    nc, in_maps = _get_compiled(x, edge_index, edge_attr, weights)
    res = bass_utils.run_bass_kernel_spmd(nc, in_maps,
                                          core_ids=list(range(N_CORES)))
    muvT = np.concatenate([res.results[c]["muvT"] for c in range(N_CORES)],
                          axis=1)
    return (np.ascontiguousarray(muvT[:LAT, :].T),
            np.ascontiguousarray(muvT[LAT:, :].T))


# revision 26
# speedup vs baseline: 8294.5217x; 1.1019x over previous
"""DRASI encoder (MLP -> GraphConv x2 -> mu/logvar heads) on 8 Trainium2 cores.

Sharding: nodes are split into 8 contiguous shards of 6250. Each core runs the
node-local MLP on its shard (transposed layout, weights as matmul lhsT), the
shards are AllGathered into a full [50000, 128] feature table in DRAM, and
each core processes the edges whose destination lies in its shard:

  - edges are sorted by dst and bucketed into 64-node "groups"; each
    (group, src-half) bucket is padded to whole 128-edge blocks, with the
    block count unified across cores (max) so all 8 cores share one program;
  - dma_gather fetches source rows from the table (int16 indices, so the
    table is addressed as two 25000-row halves);
  - a DVE-built selection matrix S_w[e, s] = w_e * (seg_e == s) turns the
    segment sum into per-block PE matmuls accumulating aggT = msg.T @ S_w
    in PSUM (features x group-nodes), evicted per bucket into an SBUF table;
  - the GraphConv linear layers + relu run on the transposed activations,
    which are PE-transposed back to natural layout only to publish the next
    AllGather table.

Outputs (mu, logvar) are computed per shard and concatenated on the host.
"""
import sys
sys.path.insert(0, '/opt/trn_rl_repo')

import numpy as np
import concourse.bass as bass
import concourse.bacc as bacc
import concourse.mybir as mybir
from concourse.tile import TileContext
from concourse.masks import make_identity
from concourse import bass_utils

P = 128
N_CORES = 8
N_NODES = 50000
IN_DIM = 512
HID = 128
LAT = 32
SHARD = N_NODES // N_CORES          # 6250
HALF = N_NODES // 2                 # 25000
W = 64                              # nodes per segment group (PSUM tile width)
MAXBLK = 48                         # max 128-edge blocks per gather chunk
HCAP = 28                           # max blocks per src-half within a chunk
N_GROUPS = (SHARD + W - 1) // W     # 98
N_TILES = [512] * (SHARD // 512) + ([SHARD % 512] if SHARD % 512 else [])
F32 = mybir.dt.float32
BF16 = mybir.dt.bfloat16
I16 = mybir.dt.int16
import ml_dtypes
NP_BF16 = ml_dtypes.bfloat16


# ---------------------------------------------------------------- host prep --

def _unified_structure(per_core_edges):
    """per_core_edges: list of (src, dst_local, w) sorted by dst_local.
    Returns (chunk_meta, per-core device arrays eidx/eseg/ew)."""
    # bucket edges per core into (group, half)
    buckets = [[[None, None] for _ in range(N_GROUPS)] for _ in range(N_CORES)]
    for c, (src, dstl, wgt) in enumerate(per_core_edges):
        grp = dstl // W
        for g in range(N_GROUPS):
            sel = grp == g
            gs, gd, gw = src[sel], dstl[sel], wgt[sel]
            hi = gs >= HALF
            for h in (0, 1):
                m = hi == bool(h)
                buckets[c][g][h] = (gs[m] - h * HALF, gd[m] - g * W, gw[m])

    # unified block count per (group, half): max over cores, >= 1 block per
    # group total so every group gets an eviction
    B = np.zeros((N_GROUPS, 2), np.int64)
    for g in range(N_GROUPS):
        for h in (0, 1):
            B[g, h] = max((buckets[c][g][h][0].shape[0] + P - 1) // P
                          for c in range(N_CORES))
        if B[g, 0] == 0 and B[g, 1] == 0:
            B[g, 0] = 1

    # pack consecutive groups into chunks of <= MAXBLK blocks, with each
    # src-half capped at HCAP (separate msgL/msgH tiles)
    chunks = []
    cur, cur_lo, cur_hi = [], 0, 0
    for g in range(N_GROUPS):
        lo, hi = int(B[g, 0]), int(B[g, 1])
        if cur and (cur_lo + lo > HCAP or cur_hi + hi > HCAP
                    or cur_lo + cur_hi + lo + hi > MAXBLK):
            chunks.append(cur)
            cur, cur_lo, cur_hi = [], 0, 0
        cur.append(g)
        cur_lo += lo
        cur_hi += hi
    if cur:
        chunks.append(cur)

    chunk_meta = []
    core_idx = [[] for _ in range(N_CORES)]
    core_seg = [[] for _ in range(N_CORES)]
    core_w = [[] for _ in range(N_CORES)]
    for groups in chunks:
        nblk_lo = int(sum(B[g, 0] for g in groups))
        nblk_hi = int(sum(B[g, 1] for g in groups))
        nblk = nblk_lo + nblk_hi
        runs = []
        b = 0
        for h in (0, 1):
            for g in groups:
                nb = int(B[g, h])
                if nb:
                    runs.append((g, h, b, b + nb))
                    b += nb
        chunk_meta.append(dict(nblk=nblk, nblk_lo=nblk_lo, runs=runs,
                               groups=list(groups)))

        for c in range(N_CORES):
            idx_flat = np.zeros(nblk * P, np.int16)
            seg_flat = np.zeros(nblk * P, np.float32)
            w_flat = np.zeros(nblk * P, np.float32)
            for (g, h, b0, b1_) in runs:
                ids, segs, ws = buckets[c][g][h]
                n = ids.shape[0]
                o = b0 * P
                idx_flat[o:o + n] = ids.astype(np.int16)
                seg_flat[o:o + n] = segs.astype(np.float32)
                w_flat[o:o + n] = ws
            idx_t = np.tile(idx_flat.reshape(nblk * 8, 16).T, (8, 1))
            seg_t = seg_flat.reshape(nblk, P).T.astype(NP_BF16).view(np.int16)
            w_t = w_flat.reshape(nblk, P).T.astype(NP_BF16).view(np.int16)
            core_idx[c].append(np.concatenate([idx_t, seg_t, w_t], axis=1))

    edata = [np.ascontiguousarray(np.concatenate(core_idx[c], axis=1))
             for c in range(N_CORES)]
    return chunk_meta, edata


# ------------------------------------------------------------- device build --

def _build(metas, idx_cols, blk_cols):
    nc = bacc.Bacc(None, target_bir_lowering=False, num_devices=N_CORES,
                   num_swdge_queues=2)

    xT = nc.dram_tensor("xT", [IN_DIM, SHARD], BF16, kind="ExternalInput")
    w1T = nc.dram_tensor("w1T", [IN_DIM, HID], BF16, kind="ExternalInput")
    b1 = nc.dram_tensor("b1", [HID, 1], F32, kind="ExternalInput")
    w2T = nc.dram_tensor("w2T", [HID, HID], BF16, kind="ExternalInput")
    b2 = nc.dram_tensor("b2", [HID, 1], F32, kind="ExternalInput")
    conv_wT = nc.dram_tensor("conv_wT", [2, 2, HID, HID], F32, kind="ExternalInput")
    conv_b = nc.dram_tensor("conv_b", [2, HID, 1], F32, kind="ExternalInput")
    headWT = nc.dram_tensor("headWT", [HID, 2 * LAT], F32, kind="ExternalInput")
    head_b = nc.dram_tensor("head_b", [2 * LAT, 1], F32, kind="ExternalInput")
    iota = nc.dram_tensor("iota", [P, W, MAXBLK], BF16, kind="ExternalInput")
    edata = nc.dram_tensor("edata", [P, idx_cols + 2 * blk_cols], I16,
                           kind="ExternalInput")
    muv_out = nc.dram_tensor("muvT", [2 * LAT, SHARD], F32, kind="ExternalOutput")

    ag_in = [nc.dram_tensor(f"ag_in{i}", [SHARD, HID], BF16) for i in range(2)]
    tables = [nc.dram_tensor(f"h_full{i}", [N_NODES, HID], BF16,
                             addr_space="Shared") for i in range(2)]

    with TileContext(nc) as tc:
        with (
            tc.tile_pool(name="const", bufs=1) as cp,
            tc.tile_pool(name="big", bufs=1) as bigp,
            tc.tile_pool(name="work", bufs=3) as wp,
            tc.tile_pool(name="ps_lin", bufs=3, space="PSUM") as ps_lin,
            tc.tile_pool(name="ps_tr", bufs=2, space="PSUM") as ps_tr,
        ):
            # ---- constants ----
            w1t_sb = [cp.tile([P, HID], BF16, tag=f"w1_{k}", name=f"w1t_{k}") for k in range(4)]
            for k in range(4):
                nc.sync.dma_start(out=w1t_sb[k][:], in_=w1T[k * P:(k + 1) * P, :])
            w2t_sb = cp.tile([P, HID], BF16, tag="w2")
            nc.sync.dma_start(out=w2t_sb[:], in_=w2T[:, :])
            cw_sb = [[cp.tile([P, HID], F32, tag=f"cw{l}{m}", name=f"cw_{l}_{m}") for m in range(2)]
                     for l in range(2)]
            for l in range(2):
                for m in range(2):
                    nc.sync.dma_start(out=cw_sb[l][m][:], in_=conv_wT[l, m, :, :])
            b1_sb = cp.tile([P, 1], F32, tag="b1")
            nc.sync.dma_start(out=b1_sb[:], in_=b1[:, :])
            b2_sb = cp.tile([P, 1], F32, tag="b2")
            nc.sync.dma_start(out=b2_sb[:], in_=b2[:, :])
            cb_sb = [cp.tile([P, 1], F32, tag=f"cb{l}", name=f"cb_{l}") for l in range(2)]
            for l in range(2):
                nc.sync.dma_start(out=cb_sb[l][:], in_=conv_b[l, :, :])
            hw_sb = cp.tile([P, 2 * LAT], F32, tag="hw")
            nc.sync.dma_start(out=hw_sb[:], in_=headWT[:, :])
            hb_sb = cp.tile([2 * LAT, 1], F32, tag="hb")
            nc.sync.dma_start(out=hb_sb[:], in_=head_b[:, :])
            iota_sb = cp.tile([P, W, MAXBLK], BF16, tag="iota")
            nc.sync.dma_start(out=iota_sb[:], in_=iota[:, :, :])
            ident = cp.tile([P, P], F32, tag="ident")
            make_identity(nc, ident[:])

            hA = bigp.tile([P, SHARD], F32, tag="hA")   # h2T, then h4T
            hB = bigp.tile([P, SHARD], F32, tag="hB")   # h3T
            aggT = bigp.tile([P, SHARD], F32, tag="aggT")

            def emit_publish_tiles(hT_tile, t_idx, n0, n1, evict="act"):
                while n0 < n1:
                    w_ = min(P, n1 - n0)
                    tr_ps = ps_tr.tile([P, P], F32, space="PSUM", tag="tr",
                                       name="trp")
                    nc.tensor.transpose(out=tr_ps[:w_, :],
                                        in_=hT_tile[:, n0:n0 + w_],
                                        identity=ident[:])
                    nat = wp.tile([P, P], BF16, tag="nat", name="nat")
                    if evict == "act":
                        nc.scalar.activation(
                            out=nat[:w_, :], in_=tr_ps[:w_, :],
                            func=mybir.ActivationFunctionType.Copy)
                    else:
                        nc.vector.tensor_copy(out=nat[:w_, :],
                                              in_=tr_ps[:w_, :])
                    nc.sync.dma_start(out=ag_in[t_idx][n0:n0 + w_, :],
                                      in_=nat[:w_, :])
                    n0 += w_

            def emit_allgather(t_idx):
                nc.gpsimd.collective_compute(
                    "AllGather", mybir.AluOpType.bypass,
                    replica_groups=[list(range(N_CORES))],
                    ins=[ag_in[t_idx][:, :]],
                    outs=[tables[t_idx][:, :]],
                )


            # ---- MLP (bf16 matmuls, f32 psum) ----
            xfp_cm = tc.tile_pool(name="xf", bufs=1)
            xfp = xfp_cm.__enter__()
            xfull = [xfp.tile([P, SHARD], BF16, tag=f"xf{k}", name=f"xf_{k}")
                     for k in range(4)]
            for k in range(4):
                eng = nc.sync if k % 2 == 0 else nc.scalar
                eng.dma_start(out=xfull[k][:],
                              in_=xT[k * P:(k + 1) * P, :])
            col = 0
            for nt in N_TILES:
                h1_ps = ps_lin.tile([P, 512], F32, space="PSUM", tag="lin")
                for k in range(4):
                    nc.tensor.matmul(out=h1_ps[:, :nt], lhsT=w1t_sb[k][:],
                                     rhs=xfull[k][:, col:col + nt],
                                     start=(k == 0), stop=(k == 3))
                h1_sb = wp.tile([P, 512], BF16, tag="h1")
                nc.scalar.activation(out=h1_sb[:, :nt], in_=h1_ps[:, :nt],
                                     func=mybir.ActivationFunctionType.Relu,
                                     bias=b1_sb[:])
                h2_ps = ps_lin.tile([P, 512], F32, space="PSUM", tag="lin")
                nc.tensor.matmul(out=h2_ps[:, :nt], lhsT=w2t_sb[:],
                                 rhs=h1_sb[:, :nt], start=True, stop=True)
                nc.scalar.activation(out=hA[:, col:col + nt], in_=h2_ps[:, :nt],
                                     func=mybir.ActivationFunctionType.Relu,
                                     bias=b2_sb[:])
                emit_publish_tiles(hA, 0, col, col + nt, evict="dve")
                col += nt

            def conv_layer(layer, hT_in, hT_out, table, pub_idx=None,
                           tile_tail=None):
                icol = 0
                for meta in metas:
                    nblk, nblk_lo = meta["nblk"], meta["nblk_lo"]
                    ed_t = wp.tile([P, MAXBLK * 10], I16, tag="ed")
                    nc.sync.dma_start(out=ed_t[:, :nblk * 10],
                                      in_=edata[:, icol:icol + nblk * 10])
                    idx_t = ed_t[:, :nblk * 8]
                    seg_t = ed_t[:, nblk * 8:nblk * 9].bitcast(BF16)
                    w_t = ed_t[:, nblk * 9:nblk * 10].bitcast(BF16)

                    msgL = msgp.tile([P, HCAP, HID], BF16, tag="msgL")
                    msgH = msgp.tile([P, HCAP, HID], BF16, tag="msgH")
                    if nblk_lo:
                        nc.gpsimd.dma_gather(
                            out_ap=msgL[:, :nblk_lo, :], in_ap=table[:HALF, :],
                            idxs_ap=idx_t[:, :nblk_lo * 8],
                            num_idxs=nblk_lo * P, num_idxs_reg=nblk_lo * P,
                            elem_size=HID, single_packet=False,
                            queue_num=0)
                    if nblk - nblk_lo:
                        nh = nblk - nblk_lo
                        nc.gpsimd.dma_gather(
                            out_ap=msgH[:, :nh, :], in_ap=table[HALF:, :],
                            idxs_ap=idx_t[:, nblk_lo * 8:nblk * 8],
                            num_idxs=nh * P, num_idxs_reg=nh * P,
                            elem_size=HID, single_packet=False,
                            queue_num=1)

                    # S_w in [p, s, block] layout: all operands' last dims are
                    # packed, which enables the DVE 2x perf mode
                    s_w = msgp.tile([P, W, MAXBLK], BF16, tag="sw")
                    nc.vector.tensor_tensor(
                        out=s_w[:, :, :nblk],
                        in0=seg_t.unsqueeze(1).to_broadcast([P, W, nblk]),
                        in1=iota_sb[:, :, :nblk],
                        op=mybir.AluOpType.is_equal)
                    nc.vector.tensor_tensor(
                        out=s_w[:, :, :nblk], in0=s_w[:, :, :nblk],
                        in1=w_t.unsqueeze(1).to_broadcast([P, W, nblk]),
                        op=mybir.AluOpType.mult)

                    # one psum + one eviction per group: a group's lo and hi
                    # runs accumulate into the same tile
                    by_group = {}
                    for (g, h, b0, b1_) in meta["runs"]:
                        by_group.setdefault(g, []).append((h, b0, b1_))
                    for g in meta["groups"]:
                        ps = ps_agg.tile([P, W], F32, space="PSUM", tag="agg")
                        blocks = [(h, b) for (h, b0, b1_) in by_group[g]
                                  for b in range(b0, b1_)]
                        for i, (h, b) in enumerate(blocks):
                            mt = msgL[:, b, :] if h == 0 else \
                                 msgH[:, b - nblk_lo, :]
                            nc.tensor.matmul(out=ps[:], lhsT=mt,
                                             rhs=s_w[:, :, b],
                                             start=(i == 0),
                                             stop=(i == len(blocks) - 1))
                        gw = min(W, SHARD - g * W)
                        nc.scalar.activation(
                            out=aggT[:, g * W:g * W + gw], in_=ps[:, :gw],
                            func=mybir.ActivationFunctionType.Copy)
                    icol += nblk * 10

                col = 0
                for nt in N_TILES:
                    ps = ps_lin.tile([P, 512], F32, space="PSUM", tag="lin")
                    nc.tensor.matmul(out=ps[:, :nt], lhsT=cw_sb[layer][0][:],
                                     rhs=aggT[:, col:col + nt],
                                     start=True, stop=False)
                    nc.tensor.matmul(out=ps[:, :nt], lhsT=cw_sb[layer][1][:],
                                     rhs=hT_in[:, col:col + nt],
                                     start=False, stop=True)
                    nc.scalar.activation(out=hT_out[:, col:col + nt],
                                         in_=ps[:, :nt],
                                         func=mybir.ActivationFunctionType.Relu,
                                         bias=cb_sb[layer][:])
                    if pub_idx is not None:
                        emit_publish_tiles(hT_out, pub_idx, col, col + nt)
                    if tile_tail is not None:
                        tile_tail(col, nt)
                    col += nt

            xfp_cm.__exit__(None, None, None)
            msgp_cm = tc.tile_pool(name="msgp", bufs=2)
            msgp = msgp_cm.__enter__()
            ps_agg_cm = tc.tile_pool(name="ps_agg", bufs=3, space="PSUM")
            ps_agg = ps_agg_cm.__enter__()
            emit_allgather(0)
            conv_layer(0, hA, hB, tables[0], pub_idx=1)
            # ---- heads fused into conv2's linear phase ----
            muvT = bigp.tile([2 * LAT, SHARD], F32, tag="muvT")

            def head_tail(col, nt):
                ps = ps_lin.tile([2 * LAT, 512], F32, space="PSUM", tag="lin",
                                 name="headps")
                nc.tensor.matmul(out=ps[:, :nt], lhsT=hw_sb[:],
                                 rhs=hA[:, col:col + nt], start=True, stop=True)
                nc.vector.tensor_tensor(
                    out=muvT[:, col:col + nt], in0=ps[:, :nt],
                    in1=hb_sb[:].to_broadcast([2 * LAT, nt]),
                    op=mybir.AluOpType.add)

            emit_allgather(1)
            conv_layer(1, hB, hA, tables[1], tile_tail=head_tail)

            msgp_cm.__exit__(None, None, None)
            ps_agg_cm.__exit__(None, None, None)
            nc.sync.dma_start(out=muv_out[:, :], in_=muvT[:])

    nc.finalize()
    return nc


# -------------------------------------------------------------------- driver --

_CACHE = {}


def _get_compiled(x, edge_index, edge_attr, weights):
    src = np.asarray(edge_index[0]).astype(np.int64)
    dst = np.asarray(edge_index[1]).astype(np.int64)
    wgt = np.asarray(edge_attr, dtype=np.float32)
    x = np.asarray(x, dtype=np.float32)

    per_core_edges = []
    for c in range(N_CORES):
        sel = (dst >= c * SHARD) & (dst < (c + 1) * SHARD)
        s, d, wv = src[sel], dst[sel] - c * SHARD, wgt[sel]
        order = np.argsort(d, kind="stable")
        per_core_edges.append((s[order], d[order], wv[order]))

    metas, edata = _unified_structure(per_core_edges)
    idx_cols = sum(m["nblk"] * 8 for m in metas)
    blk_cols = sum(m["nblk"] for m in metas)

    nc = _build(metas, idx_cols, blk_cols)

    (W1, b1, W2, b2, g1_rel_W, g1_rel_b, g1_root_W,
     g2_rel_W, g2_rel_b, g2_root_W, mu_W, mu_b, lv_W, lv_b) = [
        np.asarray(w, dtype=np.float32) for w in weights]

    conv_wT = np.stack([
        np.stack([g1_rel_W.T, g1_root_W.T]),
        np.stack([g2_rel_W.T, g2_root_W.T]),
    ]).copy()
    conv_b = np.stack([g1_rel_b[:, None], g2_rel_b[:, None]]).copy()
    headWT = np.ascontiguousarray(np.concatenate([mu_W, lv_W], axis=0).T)
    head_b = np.concatenate([mu_b, lv_b])[:, None].copy()
    iota = np.ascontiguousarray(np.broadcast_to(
        np.arange(W, dtype=np.float32)[None, :, None],
        (P, W, MAXBLK)).astype(NP_BF16))

    common = dict(
        w1T=np.ascontiguousarray(W1.T.astype(NP_BF16)), b1=b1[:, None].copy(),
        w2T=np.ascontiguousarray(W2.T.astype(NP_BF16)), b2=b2[:, None].copy(),
        conv_wT=conv_wT, conv_b=conv_b, headWT=headWT, head_b=head_b,
        iota=iota,
    )
    in_maps = []
    for c in range(N_CORES):
        m = dict(common)
        m["xT"] = np.ascontiguousarray(x[c * SHARD:(c + 1) * SHARD, :].T.astype(NP_BF16))
        m["edata"] = edata[c]
        in_maps.append(m)
    return nc, in_maps


def kernel(x, edge_index, edge_attr,
           W1, b1, W2, b2,
           g1_rel_W, g1_rel_b, g1_root_W,
           g2_rel_W, g2_rel_b, g2_root_W,
           mu_W, mu_b, lv_W, lv_b):
    weights = (W1, b1, W2, b2, g1_rel_W, g1_rel_b, g1_root_W,
               g2_rel_W, g2_rel_b, g2_root_W, mu_W, mu_b, lv_W, lv_b)
    nc, in_maps = _get_compiled(x, edge_index, edge_attr, weights)
    res = bass_utils.run_bass_kernel_spmd(nc, in_maps,
                                          core_ids=list(range(N_CORES)))
    muvT = np.concatenate([res.results[c]["muvT"] for c in range(N_CORES)],
                          axis=1)
    return (np.ascontiguousarray(muvT[:LAT, :].T),
            np.ascontiguousarray(muvT[LAT:, :].T))
